# revision 15
# baseline (speedup 1.0000x reference)
"""DNC MemoryAccess kernel for Trainium2 (Bass/Tile), data-parallel over batch.

Shapes (hardcoded): B=8, T=16, C=1024, IFACE=471, N=512, WORD=64, R=4, NW=1.
Each of the 8 cores processes one batch element; all recurrent state
(memory [64,512]T, link [512,512], usage/prec [1,512], read_w [4,512])
stays SBUF-resident across the T=16 sequential steps.
"""
import sys

sys.path.insert(0, "/opt/trn_rl_repo")

import numpy as np

import concourse.bacc as bacc
import concourse.bass as bass
import concourse.mybir as mybir
import concourse.tile as tile

F32 = mybir.dt.float32
F16 = mybir.dt.float16
AF = mybir.ActivationFunctionType
OP = mybir.AluOpType

B, T, C, IF = 8, 16, 1024, 471
N, W, R = 512, 64, 4
EPS = 1e-6
NT = N // 128  # 4 N-tiles

# iface field offsets
O_RK, O_RS, O_WK, O_WS = 0, 256, 260, 324
O_ER, O_WV, O_FG, O_AG, O_WG, O_MD = 325, 389, 453, 457, 458, 459


def build_nc():
    nc = bacc.Bacc("TRN2", target_bir_lowering=False, debug=False, num_devices=8)

    co_d = nc.declare_dram_parameter("co", [T, C], F32, isOutput=False)
    w_d = nc.declare_dram_parameter("wif", [C, IF], F32, isOutput=False)
    b_d = nc.declare_dram_parameter("bif", [1, IF], F32, isOutput=False)
    m0_d = nc.declare_dram_parameter("mem0", [N, W], F32, isOutput=False)
    ident_d = nc.declare_dram_parameter("ident", [128, 128], F32, isOutput=False)
    ones_d = nc.declare_dram_parameter("ones", [128, 128], F32, isOutput=False)
    offd_d = nc.declare_dram_parameter("offdiag", [N, N], F16, isOutput=False)
    out_d = nc.declare_dram_parameter("out", [T, R, W], F32, isOutput=True)

    with tile.TileContext(nc) as tc:
        with (
            tc.tile_pool(name="const", bufs=1) as cp,
            tc.tile_pool(name="state", bufs=2) as sp,
            tc.tile_pool(name="work", bufs=2) as wp,
            tc.tile_pool(name="psP", bufs=3, space="PSUM") as psA,
            tc.tile_pool(name="psM", bufs=3, space="PSUM") as psM,
            tc.tile_pool(name="psS", bufs=1, space="PSUM") as psS,
        ):
            _build_body(nc, tc, cp, sp, wp, psA, psM, psS,
                        co_d, w_d, b_d, m0_d, ident_d, ones_d, offd_d, out_d)
    nc.compile()
    return nc


def _build_body(nc, tc, cp, sp, wp, psA, psM, psS,
                co_d, w_d, b_d, m0_d, ident_d, ones_d, offd_d, out_d):
    V, S, P, DMA = nc.vector, nc.scalar, nc.tensor, nc.sync

    # ---------------- constants ----------------
    ident = cp.tile([128, 128], F32)
    DMA.dma_start(ident[:], ident_d[:])
    ones = cp.tile([128, 128], F32)
    DMA.dma_start(ones[:], ones_d[:])
    offd = cp.tile([128, NT, N], F16)
    for c in range(NT):
        DMA.dma_start(offd[:, c, :], offd_d[128 * c:128 * (c + 1), :])
    nege0 = cp.tile([1, N], F32)
    nc.gpsimd.memset(nege0[:], 0.0)
    nc.gpsimd.memset(nege0[0:1, 0:1], -1.0)

    # persistent per-t parse tables
    rkT = cp.tile([W, R, T], F32)
    wkT = cp.tile([W, T], F32)
    neg_er = cp.tile([W, T], F32)
    wvT = cp.tile([W, T], F32)
    rs_sb = cp.tile([R, T], F32)
    ws_sb = cp.tile([1, T], F32)
    fgF = cp.tile([1, R, T], F32)
    c1n = cp.tile([1, T], F32)
    c2 = cp.tile([1, T], F32)
    modes = cp.tile([R, 3, T], F32)

    # ---------------- prologue: iface (transient pool) ----------------
    with tc.tile_pool(name="prolog", bufs=1) as pp:
        co_sb = pp.tile([T, C], F32)
        DMA.dma_start(co_sb[:], co_d[:])
        w_sb = pp.tile([128, 8, IF], F32)
        for k in range(8):
            DMA.dma_start(w_sb[:, k, :], w_d[128 * k:128 * (k + 1), :])
        bif_sb = pp.tile([1, IF], F32)
        DMA.dma_start(bif_sb[:], b_d[:])

        coT_p = psA.tile([128, 8, T], F32, tag="p")
        for k in range(8):
            P.transpose(coT_p[:, k, :], co_sb[:, 128 * k:128 * (k + 1)],
                        ident[0:T, 0:T])
        coT = pp.tile([128, 8, T], F32)
        S.copy(coT[:], coT_p[:])

        if_p = psS.tile([T, IF], F32, tag="s")
        for k in range(8):
            P.matmul(if_p[:], coT[:, k, :], w_sb[:, k, :], start=(k == 0), stop=False)
        P.matmul(if_p[:], ones[0:1, 0:T], bif_sb[:], start=False, stop=True)
        iface = pp.tile([T, IF], F32)
        S.copy(iface[:], if_p[:])

        # field transposes -> per-t column layouts
        def tp_field(lo, hi):
            n = hi - lo
            pt = psA.tile([128, T], F32, tag="p")
            P.transpose(pt[0:n, :], iface[:, lo:hi], ident[0:T, 0:T])
            return pt

        for r in range(R):
            pt = tp_field(O_RK + W * r, O_RK + W * (r + 1))
            S.copy(rkT[:, r, :], pt[0:W, :])
        pt = tp_field(O_WK, O_WK + W)
        S.copy(wkT[:], pt[0:W, :])
        pt = tp_field(O_ER, O_ER + W)
        er_t = pp.tile([W, T], F32)
        S.activation(er_t[:], pt[0:W, :], AF.Sigmoid)
        S.mul(neg_er[:], er_t[:], -1.0)
        pt = tp_field(O_WV, O_WV + W)
        S.copy(wvT[:], pt[0:W, :])
        pt = tp_field(O_RS, O_RS + R)
        rse = pp.tile([R, T], F32)
        S.activation(rse[:], pt[0:R, :], AF.Exp)
        S.activation(rs_sb[:], rse[:], AF.Ln, bias=1.0)
        pt = tp_field(O_WS, O_WS + 1)
        wse = pp.tile([1, T], F32)
        S.activation(wse[:], pt[0:1, :], AF.Exp)
        S.activation(ws_sb[:], wse[:], AF.Ln, bias=1.0)
        fg_p = psA.tile([1, R, T], F32, tag="p")
        for r in range(R):
            P.transpose(fg_p[0:1, r, :], iface[:, O_FG + r:O_FG + r + 1],
                        ident[0:T, 0:T])
        S.activation(fgF[:], fg_p[:], AF.Sigmoid)
        agwg_p = psA.tile([1, 2, T], F32, tag="p")
        P.transpose(agwg_p[0:1, 0, :], iface[:, O_AG:O_AG + 1], ident[0:T, 0:T])
        P.transpose(agwg_p[0:1, 1, :], iface[:, O_WG:O_WG + 1], ident[0:T, 0:T])
        ag_t = pp.tile([1, T], F32)
        S.activation(ag_t[:], agwg_p[0:1, 0, :], AF.Sigmoid)
        wg_t = pp.tile([1, T], F32)
        S.activation(wg_t[:], agwg_p[0:1, 1, :], AF.Sigmoid)
        c1t = pp.tile([1, T], F32)
        V.tensor_tensor(c1t[:], ag_t[:], wg_t[:], op=OP.mult)
        S.mul(c1n[:], c1t[:], -1.0)
        V.tensor_tensor(c2[:], wg_t[:], c1t[:], op=OP.subtract)

        # modes softmax (normalize in [T,12], reorder m-major, transpose)
        me = pp.tile([T, 12], F32)
        S.activation(me[:], iface[:, O_MD:O_MD + 12], AF.Exp)
        me3 = me[:].rearrange("t (r m) -> t r m", m=3)
        msum = pp.tile([T, R], F32)
        V.tensor_tensor(msum[:], me3[:, :, 0], me3[:, :, 1], op=OP.add)
        V.tensor_tensor(msum[:], msum[:], me3[:, :, 2], op=OP.add)
        mrs = pp.tile([T, R], F32)
        V.reciprocal(mrs[:], msum[:])
        mn = pp.tile([T, 12], F32)
        mn3 = mn[:].rearrange("t (r m) -> t r m", m=3)
        for m in range(3):
            V.tensor_tensor(mn3[:, :, m], me3[:, :, m], mrs[:], op=OP.mult)
        mo = pp.tile([T, 12], F32)
        mo3 = mo[:].rearrange("t (m r) -> t m r", r=R)
        S.copy(mo3[:], mn3[:].rearrange("t r m -> t m r"))
        modes_p = psA.tile([R, 3, T], F32, tag="p")
        for m in range(3):
            P.transpose(modes_p[:, m, :], mo[:, 4 * m:4 * (m + 1)],
                        ident[0:T, 0:T])
        S.copy(modes[:], modes_p[:])

    # ---------------- initial state ----------------
    mem_nrm = sp.tile([128, NT, W], F32, tag="mem_nrm")
    for c in range(NT):
        DMA.dma_start(mem_nrm[:, c, :], m0_d[128 * c:128 * (c + 1), :])
    memT_p = psA.tile([W, N], F32, tag="p")
    for c in range(NT):
        P.transpose(memT_p[:, 128 * c:128 * (c + 1)], mem_nrm[:, c, :], ident[:])
    memT = sp.tile([W, N], F32, tag="memT")
    S.copy(memT[:], memT_p[:])

    sqm = wp.tile([W, N], F32, tag="sqm")
    S.square(sqm[:], memT[:])
    ms_p = psM.tile([1, N], F32, tag="m")
    P.matmul(ms_p[:], ones[0:W, 0:1], sqm[:])
    lnm0 = wp.tile([1, N], F32, tag="lnm")
    S.activation(lnm0[:], ms_p[:], AF.Ln)
    mnorm = sp.tile([1, N], F32, tag="mnorm")
    S.activation(mnorm[:], lnm0[:], AF.Exp, scale=0.5)

    L = sp.tile([128, NT, N], F32, tag="L")
    nc.gpsimd.memset(L[:], 0.0)
    LT0 = sp.tile([128, NT, N], F32, tag="LT")
    nc.gpsimd.memset(LT0[:], 0.0)
    u0 = sp.tile([1, N], F32, tag="u")
    nc.gpsimd.memset(u0[:], 0.0)
    rw0 = sp.tile([R, N], F32, tag="rw")
    nc.gpsimd.memset(rw0[:], 0.0)
    rwT0 = sp.tile([128, NT * R], F32, tag="rwT")
    nc.gpsimd.memset(rwT0[:], 0.0)

    out_sb = cp.tile([R, T, W], F32)

    st = dict(memT=memT, mem_nrm=mem_nrm, mnorm=mnorm, L=L, LT=LT0,
              u=u0, prec=None, rw=rw0, rwT=rwT0)

    for t in range(T):
        st = _step(nc, t, st, cp, sp, wp, psA, psM, psS,
                   ident, ones, offd, nege0, rkT, wkT, neg_er, wvT,
                   rs_sb, ws_sb, fgF, c1n, c2, modes, out_sb)

    DMA.dma_start(out_d[:].rearrange("t r w -> r t w"), out_sb[:])


def _step(nc, t, st, cp, sp, wp, psA, psM, psS,
          ident, ones, offd, nege0, rkT, wkT, neg_er, wvT,
          rs_sb, ws_sb, fgF, c1n, c2, modes, out_sb):
    V, S, P = nc.vector, nc.scalar, nc.tensor
    memT, mem_nrm, mnorm = st["memT"], st["mem_nrm"], st["mnorm"]
    L, LT, u, prec, rw, rwT = st["L"], st["LT"], st["u"], st["prec"], st["rw"], st["rwT"]

    # ---- write content weights (on memory entering the step) ----
    wdots_p = psM.tile([1, N], F32, tag="m")
    P.matmul(wdots_p[:], wkT[:, t:t + 1], memT[:])
    wk2_p = psS.tile([1, 1], F32, tag="s")
    P.matmul(wk2_p[:], wkT[:, t:t + 1], wkT[:, t:t + 1])
    wkl = wp.tile([1, 1], F32, tag="wkl")
    S.activation(wkl[:], wk2_p[:], AF.Ln)
    wkn = wp.tile([1, 1], F32, tag="wkn")
    S.activation(wkn[:], wkl[:], AF.Exp, scale=0.5)
    wden = wp.tile([1, N], F32, tag="wden")
    V.tensor_scalar(wden[:], mnorm[:], wkn[:], EPS, op0=OP.mult, op1=OP.add)
    wrden = wp.tile([1, N], F32, tag="wrden")
    V.reciprocal(wrden[:], wden[:])
    wlog = wp.tile([1, N], F32, tag="wlog")
    V.scalar_tensor_tensor(wlog[:], wdots_p[:], ws_sb[0:1, t:t + 1], wrden[:],
                           op0=OP.mult, op1=OP.mult)
    wexp = wp.tile([1, N], F32, tag="wexp")
    wsum = wp.tile([1, 1], F32, tag="wsum")
    S.activation(wexp[:], wlog[:], AF.Exp, accum_out=wsum[:])
    wrs = wp.tile([1, 1], F32, tag="wrs")
    V.reciprocal(wrs[:], wsum[:])
    wc = wp.tile([1, N], F32, tag="wc")
    V.tensor_scalar(wc[:], wexp[:], wrs[:], None, op0=OP.mult)

    # ---- allocation weighting ----
    if t == 0:
        negalloc = nege0
    else:
        u_tp = psA.tile([128, NT], F32, tag="p")
        for c in range(NT):
            P.transpose(u_tp[:, c:c + 1], u[0:1, 128 * c:128 * (c + 1)],
                        ident[0:1, 0:1])
        u_pm = wp.tile([128, NT], F32, tag="u_pm")
        S.copy(u_pm[:], u_tp[:])
        lu_pm = wp.tile([128, NT], F32, tag="lu_pm")
        S.activation(lu_pm[:], u_pm[:], AF.Ln)
        ub_p = psA.tile([128, N], F32, tag="p")
        P.matmul(ub_p[:], ones[0:1, :], u[:])
        G = wp.tile([128, NT, N], F32, tag="G", bufs=1)
        for c in range(NT):
            V.tensor_scalar(G[:, c, :], ub_p[:], u_pm[:, c:c + 1], None, op0=OP.is_gt)
        s_p = psM.tile([1, N], F32, tag="m")
        for c in range(NT):
            P.matmul(s_p[:], lu_pm[:, c:c + 1], G[:, c, :],
                     start=(c == 0), stop=(c == NT - 1))
        es = wp.tile([1, N], F32, tag="es")
        S.activation(es[:], s_p[:], AF.Exp)
        omu_a = wp.tile([1, N], F32, tag="omu")
        S.activation(omu_a[:], u[:], AF.Identity, bias=1.0, scale=-1.0)
        negalloc = wp.tile([1, N], F32, tag="negalloc")
        V.scalar_tensor_tensor(negalloc[:], omu_a[:], -1.0, es[:],
                               op0=OP.mult, op1=OP.mult)

    # ---- write weights ww ----
    t_wc = wp.tile([1, N], F32, tag="t_wc")
    V.tensor_scalar(t_wc[:], wc[:], c2[0:1, t:t + 1], None, op0=OP.mult)
    ww = wp.tile([1, N], F32, tag="ww")
    sw = wp.tile([1, 1], F32, tag="sw")
    V.scalar_tensor_tensor(ww[:], negalloc[:], c1n[0:1, t:t + 1], t_wc[:],
                           op0=OP.mult, op1=OP.add, accum_out=sw[:])

    # ---- prec update (uses prec BEFORE update; link also uses old prec) ----
    if t == 0:
        prec_n = ww  # (1-sw)*0 + ww
    else:
        omsw = wp.tile([1, 1], F32, tag="omsw")
        S.activation(omsw[:], sw[:], AF.Identity, bias=1.0, scale=-1.0)
        prec_n = sp.tile([1, N], F32, tag="prec")
        V.scalar_tensor_tensor(prec_n[:], prec[:], omsw[:], ww[:],
                               op0=OP.mult, op1=OP.add)

    # ---- usage update ----
    if t == 0:
        u_n = ww  # psi=1, u=0 -> u' = ww
    else:
        fgb_p = psA.tile([128, R], F32, tag="p")
        P.matmul(fgb_p[:], ones[0:1, :], fgF[0:1, :, t])
        yyT = wp.tile([128, NT, R], F32, tag="yyT")
        V.scalar_tensor_tensor(
            yyT[:], fgb_p[:, None, :].broadcast_to([128, NT, R]), -1.0,
            rwT[:].rearrange("p (c r) -> p c r", r=R), op0=OP.mult, op1=OP.mult)
        om = wp.tile([128, NT, R], F32, tag="om")
        S.activation(om[:], yyT[:], AF.Identity, bias=1.0)
        p1 = wp.tile([128, NT], F32, tag="p1")
        V.tensor_tensor(p1[:], om[:, :, 0], om[:, :, 1], op=OP.mult)
        p2 = wp.tile([128, NT], F32, tag="p2")
        V.tensor_tensor(p2[:], om[:, :, 2], om[:, :, 3], op=OP.mult)
        psi_pm = wp.tile([128, NT], F32, tag="psi_pm")
        V.tensor_tensor(psi_pm[:], p1[:], p2[:], op=OP.mult)
        psiT_p = psA.tile([1, N], F32, tag="p")
        for c in range(NT):
            P.transpose(psiT_p[0:1, 128 * c:128 * (c + 1)], psi_pm[:, c:c + 1],
                        ident[:])
        psi = wp.tile([1, N], F32, tag="psi")
        S.copy(psi[:], psiT_p[:])
        tn = wp.tile([1, N], F32, tag="tn")
        V.scalar_tensor_tensor(tn[:], ww[:], 1.0, omu_a[:],
                               op0=OP.subtract, op1=OP.mult)
        u_n = sp.tile([1, N], F32, tag="u")
        V.scalar_tensor_tensor(u_n[:], tn[:], 1.0, psi[:],
                               op0=OP.add, op1=OP.mult)

    # ---- memory update ----
    wwb_p = psM.tile([W, N], F32, tag="m")
    P.matmul(wwb_p[:], ones[0:1, 0:W], ww[:])
    keep = wp.tile([W, N], F32, tag="keep")
    S.activation(keep[:], wwb_p[:], AF.Identity, bias=1.0,
                 scale=neg_er[:, t:t + 1])
    m1 = wp.tile([W, N], F32, tag="m1")
    V.tensor_tensor(m1[:], memT[:], keep[:], op=OP.mult)
    memT_n = sp.tile([W, N], F32, tag="memT")
    V.scalar_tensor_tensor(memT_n[:], wwb_p[:], wvT[:, t:t + 1], m1[:],
                           op0=OP.mult, op1=OP.add)
    mem_nrm_p = psA.tile([128, NT, W], F32, tag="p")
    for c in range(NT):
        P.transpose(mem_nrm_p[:, c, :], memT_n[:, 128 * c:128 * (c + 1)],
                    ident[0:W, 0:W])
    mem_nrm_n = sp.tile([128, NT, W], F32, tag="mem_nrm")
    S.copy(mem_nrm_n[:], mem_nrm_p[:])
    sqm = wp.tile([W, N], F32, tag="sqm")
    S.square(sqm[:], memT_n[:])
    ms_p = psM.tile([1, N], F32, tag="m")
    P.matmul(ms_p[:], ones[0:W, 0:1], sqm[:])
    lnm = wp.tile([1, N], F32, tag="lnm")
    S.activation(lnm[:], ms_p[:], AF.Ln)
    mnorm_n = sp.tile([1, N], F32, tag="mnorm")
    S.activation(mnorm_n[:], lnm[:], AF.Exp, scale=0.5)

    # ---- link update ----
    if t == 0:
        L_n, LT_n = L, LT  # stays zero
    else:
        ww_tp = psA.tile([128, NT], F32, tag="p")
        for c in range(NT):
            P.transpose(ww_tp[:, c:c + 1], ww[0:1, 128 * c:128 * (c + 1)],
                        ident[0:1, 0:1])
        w_pm = wp.tile([128, NT], F32, tag="w_pm")
        S.copy(w_pm[:], ww_tp[:])
        omw_pm = wp.tile([128, NT], F32, tag="omw_pm")
        S.activation(omw_pm[:], w_pm[:], AF.Identity, bias=1.0, scale=-1.0)
        wb_p = psA.tile([128, N], F32, tag="p")
        P.matmul(wb_p[:], ones[0:1, :], ww[:])
        pb_p = psA.tile([128, N], F32, tag="p")
        P.matmul(pb_p[:], ones[0:1, :], prec[:])
        L_n = sp.tile([128, NT, N], F32, tag="L")
        for c in range(NT):
            pbm = wp.tile([128, N], F32, tag="pbm")
            V.tensor_tensor(pbm[:], pb_p[:], offd[:, c, :], op=OP.mult)
            t1 = wp.tile([128, N], F32, tag="t1")
            V.scalar_tensor_tensor(t1[:], wb_p[:], omw_pm[:, c:c + 1], L[:, c, :],
                                   op0=OP.subtract, op1=OP.mult)
            V.scalar_tensor_tensor(L_n[:, c, :], pbm[:], w_pm[:, c:c + 1], t1[:],
                                   op0=OP.mult, op1=OP.subtract)
        LT_n = sp.tile([128, NT, N], F32, tag="LT")
        for j in range(NT):
            lt_p = psA.tile([128, N], F32, tag="p")
            for i in range(NT):
                P.transpose(lt_p[:, 128 * i:128 * (i + 1)],
                            L_n[:, i, 128 * j:128 * (j + 1)], ident[:])
            S.copy(LT_n[:, j, :], lt_p[:])

    # ---- read content weights (on updated memory) ----
    rdots_p = psM.tile([R, N], F32, tag="m")
    P.matmul(rdots_p[:], rkT[:, :, t], memT_n[:])
    sqk = wp.tile([W, R], F32, tag="sqk")
    S.square(sqk[:], rkT[:, :, t])
    k2_p = psS.tile([R, 1], F32, tag="s")
    P.matmul(k2_p[:], sqk[:], ones[0:W, 0:1])
    knl = wp.tile([R, 1], F32, tag="knl")
    S.activation(knl[:], k2_p[:], AF.Ln)
    knr = wp.tile([R, 1], F32, tag="knr")
    S.activation(knr[:], knl[:], AF.Exp, scale=0.5)
    mn4_p = psM.tile([R, N], F32, tag="m")
    P.matmul(mn4_p[:], ones[0:1, 0:R], mnorm_n[:])
    rden = wp.tile([R, N], F32, tag="rden")
    V.tensor_scalar(rden[:], mn4_p[:], knr[:], EPS, op0=OP.mult, op1=OP.add)
    rrden = wp.tile([R, N], F32, tag="rrden")
    V.reciprocal(rrden[:], rden[:])
    rlog = wp.tile([R, N], F32, tag="rlog")
    V.scalar_tensor_tensor(rlog[:], rdots_p[:], rs_sb[:, t:t + 1], rrden[:],
                           op0=OP.mult, op1=OP.mult)
    rexp = wp.tile([R, N], F32, tag="rexp")
    rsum = wp.tile([R, 1], F32, tag="rsum")
    S.activation(rexp[:], rlog[:], AF.Exp, accum_out=rsum[:])
    rsr = wp.tile([R, 1], F32, tag="rsr")
    V.reciprocal(rsr[:], rsum[:])
    s1c = wp.tile([R, 1], F32, tag="s1c")
    V.tensor_tensor(s1c[:], rsr[:], modes[:, 1, t:t + 1], op=OP.mult)

    # ---- read weights ----
    rw_n = sp.tile([R, N], F32, tag="rw")
    if t == 0:
        V.tensor_scalar(rw_n[:], rexp[:], s1c[:], None, op0=OP.mult)
    else:
        bwd_p = psM.tile([R, N], F32, tag="m")
        for c in range(NT):
            P.matmul(bwd_p[:], rwT[:, R * c:R * (c + 1)], L_n[:, c, :],
                     start=(c == 0), stop=(c == NT - 1))
        fwd_p = psM.tile([R, N], F32, tag="m")
        for c in range(NT):
            P.matmul(fwd_p[:], rwT[:, R * c:R * (c + 1)], LT_n[:, c, :],
                     start=(c == 0), stop=(c == NT - 1))
        a1 = wp.tile([R, N], F32, tag="a1")
        V.tensor_scalar(a1[:], rexp[:], s1c[:], None, op0=OP.mult)
        b1 = wp.tile([R, N], F32, tag="b1")
        V.scalar_tensor_tensor(b1[:], fwd_p[:], modes[:, 2, t:t + 1], a1[:],
                               op0=OP.mult, op1=OP.add)
        V.scalar_tensor_tensor(rw_n[:], bwd_p[:], modes[:, 0, t:t + 1], b1[:],
                               op0=OP.mult, op1=OP.add)

    rwT_p = psA.tile([128, NT * R], F32, tag="p")
    for c in range(NT):
        P.transpose(rwT_p[:, R * c:R * (c + 1)], rw_n[:, 128 * c:128 * (c + 1)],
                    ident[0:R, 0:R])
    rwT_n = sp.tile([128, NT * R], F32, tag="rwT")
    S.copy(rwT_n[:], rwT_p[:])

    # ---- read words ----
    rwd_p = psS.tile([R, W], F32, tag="s")
    for c in range(NT):
        P.matmul(rwd_p[:], rwT_n[:, R * c:R * (c + 1)], mem_nrm_n[:, c, :],
                 start=(c == 0), stop=(c == NT - 1))
    S.copy(out_sb[:, t, :], rwd_p[:])

    return dict(memT=memT_n, mem_nrm=mem_nrm_n, mnorm=mnorm_n, L=L_n, LT=LT_n,
                u=u_n, prec=prec_n, rw=rw_n, rwT=rwT_n)


# ---------------------------------------------------------------------------
_NC_CACHE = {}


def _get_nc():
    if "nc" not in _NC_CACHE:
        _NC_CACHE["nc"] = build_nc()
    return _NC_CACHE["nc"]


def _consts():
    ident = np.eye(128, dtype=np.float32)
    ones = np.ones((128, 128), dtype=np.float32)
    offd = (1.0 - np.eye(N)).astype(np.float16)
    return ident, ones, offd


def make_in_maps(controller_output, W_if, b_if, memory0):
    ident, ones, offd = _consts()
    maps = []
    for b in range(B):
        maps.append({
            "co": np.ascontiguousarray(controller_output[b]),
            "wif": np.ascontiguousarray(W_if),
            "bif": np.ascontiguousarray(b_if.reshape(1, IF)),
            "mem0": np.ascontiguousarray(memory0[b]),
            "ident": ident, "ones": ones, "offdiag": offd,
        })
    return maps


def kernel(controller_output, W_if, b_if, memory0):
    from concourse.bass_utils import run_bass_kernel_spmd
    controller_output = np.asarray(controller_output, dtype=np.float32)
    W_if = np.asarray(W_if, dtype=np.float32)
    b_if = np.asarray(b_if, dtype=np.float32)
    memory0 = np.asarray(memory0, dtype=np.float32)
    nc = _get_nc()
    maps = make_in_maps(controller_output, W_if, b_if, memory0)
    res = run_bass_kernel_spmd(nc, maps, core_ids=list(range(B)))
    return np.stack([res.results[b]["out"] for b in range(B)], axis=0)


if __name__ == "__main__":
    mode = sys.argv[1] if len(sys.argv) > 1 else "sim"
    sys.path.insert(0, "/root/problem")
    import jax
    with jax.default_device(jax.devices("cpu")[0]):
        import reference
        inputs = {k: np.asarray(v) for k, v in reference.setup_inputs().items()}
        expected = np.asarray(reference.reference(**inputs))

    if mode == "sim":
        from concourse.bass_interp import CoreSim
        nc = build_nc()
        maps = make_in_maps(inputs["controller_output"], inputs["W_if"],
                            inputs["b_if"], inputs["memory0"])
        sim = CoreSim(nc)
        for k, v in maps[0].items():
            sim.tensor(k)[:] = v
        sim.simulate()
        got = sim.tensor("out").copy()
        exp = expected[0]
        err = np.abs(got - exp)
        rel = np.linalg.norm(got - exp) / (np.linalg.norm(exp) + 1e-12)
        print("sim modeled time (ns):", sim.time)
        print("max abs err:", err.max(), " rel err:", rel)
    else:
        got = kernel(**inputs)
        rel = np.linalg.norm(got - expected) / (np.linalg.norm(expected) + 1e-12)
        print("max abs err:", np.abs(got - expected).max(), " rel err:", rel)


# revision 21
# speedup vs baseline: 1.1214x; 1.1214x over previous
"""DNC MemoryAccess kernel for Trainium2 (Bass/Tile), data-parallel over batch.

Shapes (hardcoded): B=8, T=16, C=1024, IFACE=471, N=512, WORD=64, R=4, NW=1.
Each of the 8 cores processes one batch element; all recurrent state
(memory [64,512]T, link [512,512], usage/prec [1,512], read_w [4,512])
stays SBUF-resident across the T=16 sequential steps.

Precision notes: ACT-table exp has ~1e-5 max rel err, enough to flip the
DNC allocation sort on near-tied usage values. So: strengths/key-norms are
precomputed in the prologue with a full-precision polynomial exp, per-step
norms use Newton-refined rsqrt, and the three per-step exps are refined
with one ln-based correction: z = y*(1 + x - ln(y)).
"""
import sys

sys.path.insert(0, "/opt/trn_rl_repo")

import numpy as np

import concourse.bacc as bacc
import concourse.bass as bass
import concourse.mybir as mybir
import concourse.tile as tile

F32 = mybir.dt.float32
F16 = mybir.dt.float16
I32 = mybir.dt.int32
AF = mybir.ActivationFunctionType
OP = mybir.AluOpType

B, T, C, IF = 8, 16, 1024, 471
N, W, R = 512, 64, 4
NT = N // 128  # 4 N-tiles
LOG2E = 1.4426950408889634
MAGIC2 = 12582912.0 + 127.0  # round-to-int magic + exponent bias for 2^k bits
_LN2 = 0.6931471805599453
# 2^f = 1 + sum_{i>=1} EXPC[i-1] f^i  (Taylor of exp(f ln2); deg-6 err ~2e-9)
EXPC = [_LN2, _LN2**2 / 2, _LN2**3 / 6, _LN2**4 / 24, _LN2**5 / 120,
        _LN2**6 / 720]

# iface field offsets
O_RK, O_RS, O_WK, O_WS = 0, 256, 260, 324
O_ER, O_WV, O_FG, O_AG, O_WG, O_MD = 325, 389, 453, 457, 458, 459


def build_nc():
    nc = bacc.Bacc("TRN2", target_bir_lowering=False, debug=False, num_devices=8)

    co_d = nc.declare_dram_parameter("co", [T, C], F32, isOutput=False)
    w_d = nc.declare_dram_parameter("wif", [C, IF], F32, isOutput=False)
    b_d = nc.declare_dram_parameter("bif", [1, IF], F32, isOutput=False)
    m0_d = nc.declare_dram_parameter("mem0", [N, W], F32, isOutput=False)
    ident_d = nc.declare_dram_parameter("ident", [128, 128], F32, isOutput=False)
    ones_d = nc.declare_dram_parameter("ones", [128, 128], F32, isOutput=False)
    offd_d = nc.declare_dram_parameter("offdiag", [N, N], F16, isOutput=False)
    out_d = nc.declare_dram_parameter("out", [T, R, W], F32, isOutput=True)

    with tile.TileContext(nc) as tc:
        with (
            tc.tile_pool(name="const", bufs=1) as cp,
            tc.tile_pool(name="state", bufs=2) as sp,
            tc.tile_pool(name="work", bufs=2) as wp,
            tc.tile_pool(name="psP", bufs=3, space="PSUM") as psA,
            tc.tile_pool(name="psM", bufs=3, space="PSUM") as psM,
            tc.tile_pool(name="psS", bufs=1, space="PSUM") as psS,
        ):
            _build_body(nc, tc, cp, sp, wp, psA, psM, psS,
                        co_d, w_d, b_d, m0_d, ident_d, ones_d, offd_d, out_d)
    nc.compile()
    return nc


def _helpers(nc):
    V, S = nc.vector, nc.scalar

    def pexp(pool, x_ap, shape, tg, nb=1):
        """exp(x) to ~1e-7 via 2^(x*log2e): magic rounding + deg-6 poly +
        exponent-bit assembly. ~13 DVE ops; prologue/small-tensor use."""
        t_ = pool.tile(shape, F32, tag=tg + "_t", name=tg + "_t")
        V.tensor_scalar(t_[:], x_ap, LOG2E, None, op0=OP.mult)
        a_ = pool.tile(shape, F32, tag=tg + "_a", name=tg + "_a")
        V.tensor_scalar(a_[:], t_[:], MAGIC2, None, op0=OP.add)
        k_ = pool.tile(shape, F32, tag=tg + "_k", name=tg + "_k")
        V.tensor_scalar(k_[:], a_[:], MAGIC2, None, op0=OP.subtract)
        f_ = pool.tile(shape, F32, tag=tg + "_f", name=tg + "_f")
        V.tensor_tensor(f_[:], t_[:], k_[:], op=OP.subtract)
        p2 = pool.tile(shape, I32, tag=tg + "_p2", name=tg + "_p2")
        V.tensor_scalar(p2[:], a_[:].bitcast(I32), 23, None,
                        op0=OP.arith_shift_left)
        ac = [pool.tile(shape, F32, tag=tg + "_ac0", name=tg + "_ac0"),
              pool.tile(shape, F32, tag=tg + "_ac1", name=tg + "_ac1")]
        V.tensor_scalar(ac[0][:], f_[:], EXPC[5], None, op0=OP.mult)
        cur = 0
        for c_ in (EXPC[4], EXPC[3], EXPC[2], EXPC[1], EXPC[0]):
            V.scalar_tensor_tensor(ac[1 - cur][:], ac[cur][:], c_, f_[:],
                                   op0=OP.add, op1=OP.mult)
            cur = 1 - cur
        y_ = pool.tile(shape, F32, tag=tg + "_y", name=tg + "_y")
        V.scalar_tensor_tensor(y_[:], ac[cur][:], 1.0, p2[:].bitcast(F32),
                               op0=OP.add, op1=OP.mult)
        return y_

    def softplus_precise(pool, x_ap, shape, tg):
        """ln(1+e^x) with table-ln seed + one Newton step (via pexp)."""
        e_ = pexp(pool, x_ap, shape, tg + "e")
        w_ = pool.tile(shape, F32, tag=tg + "_w", name=tg + "_w")
        V.tensor_scalar(w_[:], e_[:], 1.0, None, op0=OP.add)
        z_ = pool.tile(shape, F32, tag=tg + "_z", name=tg + "_z")
        S.activation(z_[:], w_[:], AF.Ln)
        nz = pool.tile(shape, F32, tag=tg + "_nz", name=tg + "_nz")
        S.mul(nz[:], z_[:], -1.0)
        e2 = pexp(pool, nz[:], shape, tg + "e2")
        m_ = pool.tile(shape, F32, tag=tg + "_m", name=tg + "_m")
        V.tensor_tensor(m_[:], w_[:], e2[:], op=OP.mult)
        o_ = pool.tile(shape, F32, tag=tg + "_o", name=tg + "_o")
        V.scalar_tensor_tensor(o_[:], m_[:], -1.0, z_[:], op0=OP.add, op1=OP.add)
        return o_

    def rsqrt_refined(pool, x_ap, shape, tg, iters=1, nb=1):
        """rsqrt(x): ACT-table seed exp(-0.5 ln x) + Newton (no division)."""
        l_ = pool.tile(shape, F32, tag=tg + "_l", name=tg + "_l", bufs=nb)
        S.activation(l_[:], x_ap, AF.Ln)
        y_ = pool.tile(shape, F32, tag=tg + "_y", name=tg + "_y", bufs=nb)
        S.activation(y_[:], l_[:], AF.Exp, scale=-0.5)
        for i in range(iters):
            s_ = pool.tile(shape, F32, tag=tg + f"_s{i}", name=tg + f"_s{i}", bufs=nb)
            V.tensor_tensor(s_[:], y_[:], y_[:], op=OP.mult)
            t_ = pool.tile(shape, F32, tag=tg + f"_t{i}", name=tg + f"_t{i}", bufs=nb)
            V.tensor_tensor(t_[:], x_ap, s_[:], op=OP.mult)
            h_ = pool.tile(shape, F32, tag=tg + f"_h{i}", name=tg + f"_h{i}", bufs=nb)
            V.tensor_scalar(h_[:], t_[:], -0.5, 1.5, op0=OP.mult, op1=OP.add)
            y2 = pool.tile(shape, F32, tag=tg + f"_y{i}", name=tg + f"_y{i}", bufs=nb)
            V.tensor_tensor(y2[:], y_[:], h_[:], op=OP.mult)
            y_ = y2
        return y_

    def exp_refined(pool, x_ap, shape, tg, out, accum_out=None):
        """exp(x) = table seed y, then out = y*(1 + x - ln(y))."""
        y_ = pool.tile(shape, F32, tag=tg + "_y", name=tg + "_y", bufs=1)
        S.activation(y_[:], x_ap, AF.Exp)
        ly = pool.tile(shape, F32, tag=tg + "_ly", name=tg + "_ly", bufs=1)
        S.activation(ly[:], y_[:], AF.Ln)
        d_ = pool.tile(shape, F32, tag=tg + "_d", name=tg + "_d", bufs=1)
        V.tensor_tensor(d_[:], x_ap, ly[:], op=OP.subtract)
        V.scalar_tensor_tensor(out, d_[:], 1.0, y_[:], op0=OP.add,
                               op1=OP.mult, accum_out=accum_out)
        return out

    return pexp, softplus_precise, rsqrt_refined, exp_refined


def _build_body(nc, tc, cp, sp, wp, psA, psM, psS,
                co_d, w_d, b_d, m0_d, ident_d, ones_d, offd_d, out_d):
    V, S, P, DMA = nc.vector, nc.scalar, nc.tensor, nc.sync
    pexp, softplus_precise, rsqrt_refined, exp_refined = _helpers(nc)

    # ---------------- constants ----------------
    ident = cp.tile([128, 128], F32)
    DMA.dma_start(ident[:], ident_d[:])
    ones = cp.tile([128, 128], F32)
    DMA.dma_start(ones[:], ones_d[:])
    offd = cp.tile([128, NT, N], F16)
    for c in range(NT):
        DMA.dma_start(offd[:, c, :], offd_d[128 * c:128 * (c + 1), :])
    nege0 = cp.tile([1, N], F32)
    nc.gpsimd.memset(nege0[:], 0.0)
    nc.gpsimd.memset(nege0[0:1, 0:1], -1.0)

    # persistent per-t parse tables
    rkT = cp.tile([W, R, T], F32)
    wkT = cp.tile([W, T], F32)
    neg_er = cp.tile([W, T], F32)
    wvT = cp.tile([W, T], F32)
    bkw = cp.tile([1, T], F32)       # softplus(ws)/||wk||
    bkr = cp.tile([R, T], F32)       # softplus(rs)/||rk|| (partition-major)
    fgF = cp.tile([1, R, T], F32)
    c1n = cp.tile([1, T], F32)
    c2 = cp.tile([1, T], F32)
    modes = cp.tile([R, 3, T], F32)

    # ---------------- prologue: iface (transient pool) ----------------
    with tc.tile_pool(name="prolog", bufs=1) as pp:
        co_sb = pp.tile([T, C], F32)
        DMA.dma_start(co_sb[:], co_d[:])
        bif_sb = pp.tile([1, IF], F32)
        DMA.dma_start(bif_sb[:], b_d[:])

        coT_p = psA.tile([128, 8, T], F32, tag="p")
        for k in range(8):
            P.transpose(coT_p[:, k, :], co_sb[:, 128 * k:128 * (k + 1)],
                        ident[0:T, 0:T])
        coT = pp.tile([128, 8, T], F32)
        S.copy(coT[:], coT_p[:])

        if_p = psS.tile([T, IF], F32, tag="s")
        for h in range(2):
            w_sb = pp.tile([128, 4, IF], F32, tag="w_sb", name=f"w_sb{h}")
            for k in range(4):
                DMA.dma_start(w_sb[:, k, :],
                              w_d[128 * (4 * h + k):128 * (4 * h + k + 1), :])
            for k in range(4):
                P.matmul(if_p[:], coT[:, 4 * h + k, :], w_sb[:, k, :],
                         start=(h == 0 and k == 0), stop=False)
        P.matmul(if_p[:], ones[0:1, 0:T], bif_sb[:], start=False, stop=True)
        iface = pp.tile([T, IF], F32)
        S.copy(iface[:], if_p[:])

        # field transposes -> per-t column layouts
        def tp_field(lo, hi):
            n = hi - lo
            pt = psA.tile([128, T], F32, tag="p")
            P.transpose(pt[0:n, :], iface[:, lo:hi], ident[0:T, 0:T])
            return pt

        for r in range(R):
            pt = tp_field(O_RK + W * r, O_RK + W * (r + 1))
            S.copy(rkT[:, r, :], pt[0:W, :])
        pt = tp_field(O_WK, O_WK + W)
        S.copy(wkT[:], pt[0:W, :])
        pt = tp_field(O_ER, O_ER + W)
        er_t = pp.tile([W, T], F32)
        S.activation(er_t[:], pt[0:W, :], AF.Sigmoid)
        S.mul(neg_er[:], er_t[:], -1.0)
        pt = tp_field(O_WV, O_WV + W)
        S.copy(wvT[:], pt[0:W, :])

        # strengths / ||k|| folded: bkw, bkr  (free-layout pipeline)
        rsF_p = psA.tile([1, R, T], F32, tag="p")
        for r in range(R):
            P.transpose(rsF_p[0:1, r, :], iface[:, O_RS + r:O_RS + r + 1],
                        ident[0:T, 0:T])
        rsF = pp.tile([1, R, T], F32)
        S.copy(rsF[:], rsF_p[:])
        wsF_p = psA.tile([1, T], F32, tag="p")
        P.transpose(wsF_p[:], iface[:, O_WS:O_WS + 1], ident[0:T, 0:T])
        wsF = pp.tile([1, T], F32)
        S.copy(wsF[:], wsF_p[:])
        rs_pre = softplus_precise(pp, rsF[:].rearrange("o r t -> o (r t)"),
                                  [1, R * T], "rsp")
        ws_pre = softplus_precise(pp, wsF[:], [1, T], "wsp")

        sqw = pp.tile([W, T], F32)
        S.square(sqw[:], wkT[:])
        wk2_p = psM.tile([1, T], F32, tag="m")
        P.matmul(wk2_p[:], ones[0:W, 0:1], sqw[:])
        wkr = rsqrt_refined(pp, wk2_p[:], [1, T], "wkr", iters=2)
        V.tensor_tensor(bkw[:], ws_pre[:], wkr[:], op=OP.mult)

        sqr = pp.tile([W, R, T], F32)
        S.square(sqr[:], rkT[:])
        rk2_p = psM.tile([1, R * T], F32, tag="m")
        P.matmul(rk2_p[:], ones[0:W, 0:1], sqr[:].rearrange("w r t -> w (r t)"))
        rkr = rsqrt_refined(pp, rk2_p[:], [1, R * T], "rkr", iters=2)
        bkrF = pp.tile([1, R, T], F32)
        V.tensor_tensor(bkrF[:].rearrange("o r t -> o (r t)"), rs_pre[:],
                        rkr[:], op=OP.mult)
        bkr_p = psA.tile([R, T], F32, tag="p")
        for t in range(T):
            P.transpose(bkr_p[:, t:t + 1], bkrF[0:1, :, t], ident[0:1, 0:1])
        S.copy(bkr[:], bkr_p[:])

        # gates
        fg_p = psA.tile([1, R, T], F32, tag="p")
        for r in range(R):
            P.transpose(fg_p[0:1, r, :], iface[:, O_FG + r:O_FG + r + 1],
                        ident[0:T, 0:T])
        S.activation(fgF[:], fg_p[:], AF.Sigmoid)
        agwg_p = psA.tile([1, 2, T], F32, tag="p")
        P.transpose(agwg_p[0:1, 0, :], iface[:, O_AG:O_AG + 1], ident[0:T, 0:T])
        P.transpose(agwg_p[0:1, 1, :], iface[:, O_WG:O_WG + 1], ident[0:T, 0:T])
        ag_t = pp.tile([1, T], F32)
        S.activation(ag_t[:], agwg_p[0:1, 0, :], AF.Sigmoid)
        wg_t = pp.tile([1, T], F32)
        S.activation(wg_t[:], agwg_p[0:1, 1, :], AF.Sigmoid)
        c1t = pp.tile([1, T], F32)
        V.tensor_tensor(c1t[:], ag_t[:], wg_t[:], op=OP.mult)
        S.mul(c1n[:], c1t[:], -1.0)
        V.tensor_tensor(c2[:], wg_t[:], c1t[:], op=OP.subtract)

        # modes softmax (precise exp; normalize in [T,12]; m-major; transpose)
        me = pexp(pp, iface[:, O_MD:O_MD + 12], [T, 12], "me")
        me3 = me[:].rearrange("t (r m) -> t r m", m=3)
        msum = pp.tile([T, R], F32)
        V.tensor_tensor(msum[:], me3[:, :, 0], me3[:, :, 1], op=OP.add)
        V.tensor_tensor(msum[:], msum[:], me3[:, :, 2], op=OP.add)
        mrs = pp.tile([T, R], F32)
        V.reciprocal(mrs[:], msum[:])
        mn = pp.tile([T, 12], F32)
        mn3 = mn[:].rearrange("t (r m) -> t r m", m=3)
        for m in range(3):
            V.tensor_tensor(mn3[:, :, m], me3[:, :, m], mrs[:], op=OP.mult)
        mo = pp.tile([T, 12], F32)
        mo3 = mo[:].rearrange("t (m r) -> t m r", r=R)
        S.copy(mo3[:], mn3[:].rearrange("t r m -> t m r"))
        modes_p = psA.tile([R, 3, T], F32, tag="p")
        for m in range(3):
            P.transpose(modes_p[:, m, :], mo[:, 4 * m:4 * (m + 1)],
                        ident[0:T, 0:T])
        S.copy(modes[:], modes_p[:])

    # ---------------- initial state ----------------
    mem_nrm = sp.tile([128, NT, W], F32, tag="mem_nrm")
    for c in range(NT):
        DMA.dma_start(mem_nrm[:, c, :], m0_d[128 * c:128 * (c + 1), :])
    memT_p = psA.tile([W, N], F32, tag="p")
    for c in range(NT):
        P.transpose(memT_p[:, 128 * c:128 * (c + 1)], mem_nrm[:, c, :], ident[:])
    memT = sp.tile([W, N], F32, tag="memT")
    S.copy(memT[:], memT_p[:])

    sqm = wp.tile([W, N], F32, tag="sqm", bufs=1)
    S.square(sqm[:], memT[:])
    ms_p = psM.tile([1, N], F32, tag="m")
    P.matmul(ms_p[:], ones[0:W, 0:1], sqm[:])
    w_rs = rsqrt_refined(wp, ms_p[:], [1, N], "w1", iters=1)
    mnorm = sp.tile([1, N], F32, tag="mnorm")
    V.tensor_copy(mnorm[:], w_rs[:])

    L = sp.tile([128, NT, N], F32, tag="L")
    nc.gpsimd.memset(L[:], 0.0)
    LT0 = sp.tile([128, NT, N], F32, tag="LT")
    nc.gpsimd.memset(LT0[:], 0.0)
    u0 = sp.tile([1, N], F32, tag="u")
    nc.gpsimd.memset(u0[:], 0.0)
    rw0 = sp.tile([R, N], F32, tag="rw")
    nc.gpsimd.memset(rw0[:], 0.0)
    rwT0 = sp.tile([128, NT * R], F32, tag="rwT")
    nc.gpsimd.memset(rwT0[:], 0.0)

    out_sb = cp.tile([R, T, W], F32)

    st = dict(memT=memT, mem_nrm=mem_nrm, mnorm=mnorm, L=L, LT=LT0,
              u=u0, prec=None, rw=rw0, rwT=rwT0)

    for t in range(T):
        st = _step(nc, t, st, cp, sp, wp, psA, psM, psS,
                   ident, ones, offd, nege0, rkT, wkT, neg_er, wvT,
                   bkw, bkr, fgF, c1n, c2, modes, out_sb,
                   rsqrt_refined, exp_refined)

    DMA.dma_start(out_d[:].rearrange("t r w -> r t w"), out_sb[:])


def _step(nc, t, st, cp, sp, wp, psA, psM, psS,
          ident, ones, offd, nege0, rkT, wkT, neg_er, wvT,
          bkw, bkr, fgF, c1n, c2, modes, out_sb,
          rsqrt_refined, exp_refined):
    V, S, P = nc.vector, nc.scalar, nc.tensor
    memT, mem_nrm, mnorm = st["memT"], st["mem_nrm"], st["mnorm"]
    L, LT, u, prec, rw, rwT = st["L"], st["LT"], st["u"], st["prec"], st["rw"], st["rwT"]
    last = (t == T - 1)

    # ---- write content weights (on memory entering the step) ----
    # mnorm state holds rsqrt(sum mem^2); bkw = softplus(ws)/||wk||.
    wdots_p = psM.tile([1, N], F32, tag="m")
    P.matmul(wdots_p[:], wkT[:, t:t + 1], memT[:])
    wlog = wp.tile([1, N], F32, tag="wlog")
    V.scalar_tensor_tensor(wlog[:], wdots_p[:], bkw[0:1, t:t + 1], mnorm[:],
                           op0=OP.mult, op1=OP.mult)
    wsum = wp.tile([1, 1], F32, tag="wsum")
    wexp = wp.tile([1, N], F32, tag="wexp")
    exp_refined(wp, wlog[:], [1, N], "wex", wexp[:], accum_out=wsum[:])
    wrs = wp.tile([1, 1], F32, tag="wrs")
    V.reciprocal(wrs[:], wsum[:])
    wc = wp.tile([1, N], F32, tag="wc")
    V.tensor_scalar(wc[:], wexp[:], wrs[:], None, op0=OP.mult)

    # ---- allocation weighting ----
    if t == 0:
        negalloc = nege0
        omu_a = None
    else:
        u_tp = psA.tile([128, NT], F32, tag="p")
        for c in range(NT):
            P.transpose(u_tp[:, c:c + 1], u[0:1, 128 * c:128 * (c + 1)],
                        ident[0:1, 0:1])
        u_pm = wp.tile([128, NT], F32, tag="u_pm")
        S.copy(u_pm[:], u_tp[:])
        lu_pm = wp.tile([128, NT], F32, tag="lu_pm")
        S.activation(lu_pm[:], u_pm[:], AF.Ln)
        ub_p = psA.tile([128, N], F32, tag="p")
        P.matmul(ub_p[:], ones[0:1, :], u[:])
        G = wp.tile([128, NT, N], F32, tag="G", bufs=1)
        for c in range(NT):
            V.tensor_scalar(G[:, c, :], ub_p[:], u_pm[:, c:c + 1], None, op0=OP.is_gt)
        s_p = psM.tile([1, N], F32, tag="m")
        for c in range(NT):
            P.matmul(s_p[:], lu_pm[:, c:c + 1], G[:, c, :],
                     start=(c == 0), stop=(c == NT - 1))
        s_c = wp.tile([1, N], F32, tag="s_c")
        V.tensor_scalar(s_c[:], s_p[:], -80.0, None, op0=OP.max)
        es = wp.tile([1, N], F32, tag="es")
        exp_refined(wp, s_c[:], [1, N], "wex", es[:])
        omu_a = wp.tile([1, N], F32, tag="omu")
        S.activation(omu_a[:], u[:], AF.Identity, bias=1.0, scale=-1.0)
        negalloc = wp.tile([1, N], F32, tag="negalloc")
        V.scalar_tensor_tensor(negalloc[:], omu_a[:], -1.0, es[:],
                               op0=OP.mult, op1=OP.mult)

    # ---- write weights ww ----
    t_wc = wp.tile([1, N], F32, tag="t_wc")
    V.tensor_scalar(t_wc[:], wc[:], c2[0:1, t:t + 1], None, op0=OP.mult)
    ww = wp.tile([1, N], F32, tag="ww")
    sw = wp.tile([1, 1], F32, tag="sw")
    V.scalar_tensor_tensor(ww[:], negalloc[:], c1n[0:1, t:t + 1], t_wc[:],
                           op0=OP.mult, op1=OP.add, accum_out=sw[:])

    # ---- prec update (uses prec BEFORE update; link also uses old prec) ----
    if t == 0:
        prec_n = ww  # (1-sw)*0 + ww
    elif last:
        prec_n = None
    else:
        omsw = wp.tile([1, 1], F32, tag="omsw")
        S.activation(omsw[:], sw[:], AF.Identity, bias=1.0, scale=-1.0)
        prec_n = sp.tile([1, N], F32, tag="prec")
        V.scalar_tensor_tensor(prec_n[:], prec[:], omsw[:], ww[:],
                               op0=OP.mult, op1=OP.add)

    # ---- usage update ----
    if t == 0:
        u_n = ww  # psi=1, u=0 -> u' = ww
    elif last:
        u_n = None
    else:
        fgb_p = psA.tile([128, R], F32, tag="p")
        P.matmul(fgb_p[:], ones[0:1, :], fgF[0:1, :, t])
        yyT = wp.tile([128, NT, R], F32, tag="yyT")
        V.scalar_tensor_tensor(
            yyT[:], fgb_p[:, None, :].broadcast_to([128, NT, R]), -1.0,
            rwT[:].rearrange("p (c r) -> p c r", r=R), op0=OP.mult, op1=OP.mult)
        om = wp.tile([128, NT, R], F32, tag="om")
        S.activation(om[:], yyT[:], AF.Identity, bias=1.0)
        p1 = wp.tile([128, NT], F32, tag="p1")
        V.tensor_tensor(p1[:], om[:, :, 0], om[:, :, 1], op=OP.mult)
        p2 = wp.tile([128, NT], F32, tag="p2")
        V.tensor_tensor(p2[:], om[:, :, 2], om[:, :, 3], op=OP.mult)
        psi_pm = wp.tile([128, NT], F32, tag="psi_pm")
        V.tensor_tensor(psi_pm[:], p1[:], p2[:], op=OP.mult)
        psiT_p = psA.tile([1, N], F32, tag="p")
        for c in range(NT):
            P.transpose(psiT_p[0:1, 128 * c:128 * (c + 1)], psi_pm[:, c:c + 1],
                        ident[:])
        psi = wp.tile([1, N], F32, tag="psi")
        S.copy(psi[:], psiT_p[:])
        tn = wp.tile([1, N], F32, tag="tn")
        V.scalar_tensor_tensor(tn[:], ww[:], 1.0, omu_a[:],
                               op0=OP.subtract, op1=OP.mult)
        u_n = sp.tile([1, N], F32, tag="u")
        V.scalar_tensor_tensor(u_n[:], tn[:], 1.0, psi[:],
                               op0=OP.add, op1=OP.mult)

    # ---- memory update ----
    wwb_p = psM.tile([W, N], F32, tag="m")
    P.matmul(wwb_p[:], ones[0:1, 0:W], ww[:])
    keep = wp.tile([W, N], F32, tag="keep", bufs=1)
    S.activation(keep[:], wwb_p[:], AF.Identity, bias=1.0,
                 scale=neg_er[:, t:t + 1])
    m1 = wp.tile([W, N], F32, tag="m1", bufs=1)
    V.tensor_tensor(m1[:], memT[:], keep[:], op=OP.mult)
    memT_n = sp.tile([W, N], F32, tag="memT")
    V.scalar_tensor_tensor(memT_n[:], wwb_p[:], wvT[:, t:t + 1], m1[:],
                           op0=OP.mult, op1=OP.add)
    mem_nrm_p = psA.tile([128, NT, W], F32, tag="p")
    for c in range(NT):
        P.transpose(mem_nrm_p[:, c, :], memT_n[:, 128 * c:128 * (c + 1)],
                    ident[0:W, 0:W])
    mem_nrm_n = sp.tile([128, NT, W], F32, tag="mem_nrm")
    S.copy(mem_nrm_n[:], mem_nrm_p[:])
    sqm = wp.tile([W, N], F32, tag="sqm", bufs=1)
    S.square(sqm[:], memT_n[:])
    ms_p = psM.tile([1, N], F32, tag="m")
    P.matmul(ms_p[:], ones[0:W, 0:1], sqm[:])
    w_rs = rsqrt_refined(wp, ms_p[:], [1, N], "w1", iters=1)
    mnorm_n = sp.tile([1, N], F32, tag="mnorm")
    V.tensor_copy(mnorm_n[:], w_rs[:])

    # ---- link update ----
    if t == 0:
        L_n, LT_n = L, LT  # stays zero
    else:
        ww_tp = psA.tile([128, NT], F32, tag="p")
        for c in range(NT):
            P.transpose(ww_tp[:, c:c + 1], ww[0:1, 128 * c:128 * (c + 1)],
                        ident[0:1, 0:1])
        w_pm = wp.tile([128, NT], F32, tag="w_pm")
        S.copy(w_pm[:], ww_tp[:])
        omw_pm = wp.tile([128, NT], F32, tag="omw_pm")
        S.activation(omw_pm[:], w_pm[:], AF.Identity, bias=1.0, scale=-1.0)
        wb_p = psA.tile([128, N], F32, tag="p")
        P.matmul(wb_p[:], ones[0:1, :], ww[:])
        pb_p = psA.tile([128, N], F32, tag="p")
        P.matmul(pb_p[:], ones[0:1, :], prec[:])
        L_n = sp.tile([128, NT, N], F32, tag="L")
        for c in range(NT):
            pbm = wp.tile([128, N], F32, tag="pbm")
            V.tensor_tensor(pbm[:], pb_p[:], offd[:, c, :], op=OP.mult)
            t1 = wp.tile([128, N], F32, tag="t1")
            V.scalar_tensor_tensor(t1[:], wb_p[:], omw_pm[:, c:c + 1], L[:, c, :],
                                   op0=OP.subtract, op1=OP.mult)
            V.scalar_tensor_tensor(L_n[:, c, :], pbm[:], w_pm[:, c:c + 1], t1[:],
                                   op0=OP.mult, op1=OP.subtract)
        LT_n = sp.tile([128, NT, N], F32, tag="LT")
        for j in range(NT):
            lt_p = psA.tile([128, N], F32, tag="p")
            for i in range(NT):
                P.transpose(lt_p[:, 128 * i:128 * (i + 1)],
                            L_n[:, i, 128 * j:128 * (j + 1)], ident[:])
            S.copy(LT_n[:, j, :], lt_p[:])

    # ---- read content weights (on updated memory) ----
    rdots_p = psM.tile([R, N], F32, tag="m")
    P.matmul(rdots_p[:], rkT[:, :, t], memT_n[:])
    mn4_p = psM.tile([R, N], F32, tag="m")
    P.matmul(mn4_p[:], ones[0:1, 0:R], mnorm_n[:])
    w4_sb = wp.tile([R, N], F32, tag="w4")
    S.copy(w4_sb[:], mn4_p[:])
    rlog = wp.tile([R, N], F32, tag="rlog")
    V.scalar_tensor_tensor(rlog[:], rdots_p[:], bkr[:, t:t + 1], w4_sb[:],
                           op0=OP.mult, op1=OP.mult)
    rsum = wp.tile([R, 1], F32, tag="rsum")
    rexp = wp.tile([R, N], F32, tag="rexp")
    exp_refined(wp, rlog[:], [R, N], "rex", rexp[:], accum_out=rsum[:])
    rsr = wp.tile([R, 1], F32, tag="rsr")
    V.reciprocal(rsr[:], rsum[:])
    s1c = wp.tile([R, 1], F32, tag="s1c")
    V.tensor_tensor(s1c[:], rsr[:], modes[:, 1, t:t + 1], op=OP.mult)

    # ---- read weights ----
    rw_n = sp.tile([R, N], F32, tag="rw")
    if t == 0:
        V.tensor_scalar(rw_n[:], rexp[:], s1c[:], None, op0=OP.mult)
    else:
        bwd_p = psM.tile([R, N], F32, tag="m")
        for c in range(NT):
            P.matmul(bwd_p[:], rwT[:, R * c:R * (c + 1)], L_n[:, c, :],
                     start=(c == 0), stop=(c == NT - 1))
        fwd_p = psM.tile([R, N], F32, tag="m")
        for c in range(NT):
            P.matmul(fwd_p[:], rwT[:, R * c:R * (c + 1)], LT_n[:, c, :],
                     start=(c == 0), stop=(c == NT - 1))
        a1 = wp.tile([R, N], F32, tag="a1")
        V.tensor_scalar(a1[:], rexp[:], s1c[:], None, op0=OP.mult)
        b1 = wp.tile([R, N], F32, tag="b1")
        V.scalar_tensor_tensor(b1[:], fwd_p[:], modes[:, 2, t:t + 1], a1[:],
                               op0=OP.mult, op1=OP.add)
        V.scalar_tensor_tensor(rw_n[:], bwd_p[:], modes[:, 0, t:t + 1], b1[:],
                               op0=OP.mult, op1=OP.add)

    rwT_p = psA.tile([128, NT * R], F32, tag="p")
    for c in range(NT):
        P.transpose(rwT_p[:, R * c:R * (c + 1)], rw_n[:, 128 * c:128 * (c + 1)],
                    ident[0:R, 0:R])
    rwT_n = sp.tile([128, NT * R], F32, tag="rwT")
    S.copy(rwT_n[:], rwT_p[:])

    # ---- read words ----
    rwd_p = psS.tile([R, W], F32, tag="s")
    for c in range(NT):
        P.matmul(rwd_p[:], rwT_n[:, R * c:R * (c + 1)], mem_nrm_n[:, c, :],
                 start=(c == 0), stop=(c == NT - 1))
    S.copy(out_sb[:, t, :], rwd_p[:])

    return dict(memT=memT_n, mem_nrm=mem_nrm_n, mnorm=mnorm_n, L=L_n, LT=LT_n,
                u=u_n, prec=prec_n, rw=rw_n, rwT=rwT_n)


# ---------------------------------------------------------------------------
_NC_CACHE = {}


def _get_nc():
    if "nc" not in _NC_CACHE:
        _NC_CACHE["nc"] = build_nc()
    return _NC_CACHE["nc"]


def _consts():
    ident = np.eye(128, dtype=np.float32)
    ones = np.ones((128, 128), dtype=np.float32)
    offd = (1.0 - np.eye(N)).astype(np.float16)
    return ident, ones, offd


def make_in_maps(controller_output, W_if, b_if, memory0):
    ident, ones, offd = _consts()
    maps = []
    for b in range(B):
        maps.append({
            "co": np.ascontiguousarray(controller_output[b]),
            "wif": np.ascontiguousarray(W_if),
            "bif": np.ascontiguousarray(b_if.reshape(1, IF)),
            "mem0": np.ascontiguousarray(memory0[b]),
            "ident": ident, "ones": ones, "offdiag": offd,
        })
    return maps


def kernel(controller_output, W_if, b_if, memory0):
    from concourse.bass_utils import run_bass_kernel_spmd
    controller_output = np.asarray(controller_output, dtype=np.float32)
    W_if = np.asarray(W_if, dtype=np.float32)
    b_if = np.asarray(b_if, dtype=np.float32)
    memory0 = np.asarray(memory0, dtype=np.float32)
    nc = _get_nc()
    maps = make_in_maps(controller_output, W_if, b_if, memory0)
    res = run_bass_kernel_spmd(nc, maps, core_ids=list(range(B)))
    return np.stack([res.results[b]["out"] for b in range(B)], axis=0)


if __name__ == "__main__":
    mode = sys.argv[1] if len(sys.argv) > 1 else "sim"
    sys.path.insert(0, "/root/problem")
    import jax
    with jax.default_device(jax.devices("cpu")[0]):
        import reference
        inputs = {k: np.asarray(v) for k, v in reference.setup_inputs().items()}
        expected = np.asarray(reference.reference(**inputs))

    if mode == "sim":
        from concourse.bass_interp import CoreSim
        nc = build_nc()
        maps = make_in_maps(inputs["controller_output"], inputs["W_if"],
                            inputs["b_if"], inputs["memory0"])
        sim = CoreSim(nc)
        for k, v in maps[0].items():
            sim.tensor(k)[:] = v
        sim.simulate()
        got = sim.tensor("out").copy()
        exp = expected[0]
        err = np.abs(got - exp)
        rel = np.linalg.norm(got - exp) / (np.linalg.norm(exp) + 1e-12)
        print("sim modeled time (ns):", sim.time)
        print("max abs err:", err.max(), " rel err:", rel)
    else:
        got = kernel(**inputs)
        rel = np.linalg.norm(got - expected) / (np.linalg.norm(expected) + 1e-12)
        print("max abs err:", np.abs(got - expected).max(), " rel err:", rel)


# revision 33
# speedup vs baseline: 1232.9048x; 1099.3863x over previous
"""DNC MemoryAccess kernel for Trainium2 (Bass/Tile), data-parallel over batch.

Shapes (hardcoded): B=8, T=16, C=1024, IFACE=471, N=512, WORD=64, R=4, NW=1.
Each of the 8 cores processes one batch element; all recurrent state
(memory [64,512]T, link [512,512], usage/prec [1,512], read_w [4,512])
stays SBUF-resident across the T=16 sequential steps.

Precision notes: ACT-table exp has ~1e-5 max rel err, enough to flip the
DNC allocation sort on near-tied usage values. So: strengths/key-norms are
precomputed in the prologue with a full-precision polynomial exp, per-step
norms use Newton-refined rsqrt, and the three per-step exps are refined
with one ln-based correction: z = y*(1 + x - ln(y)).
"""
import sys

sys.path.insert(0, "/opt/trn_rl_repo")

import numpy as np

import concourse.bacc as bacc
import concourse.bass as bass
import concourse.mybir as mybir
import concourse.tile as tile

F32 = mybir.dt.float32
F16 = mybir.dt.float16
I32 = mybir.dt.int32
AF = mybir.ActivationFunctionType
OP = mybir.AluOpType

B, T, C, IF = 8, 16, 1024, 471
N, W, R = 512, 64, 4
NT = N // 128  # 4 N-tiles
LOG2E = 1.4426950408889634
MAGIC2 = 12582912.0 + 127.0  # round-to-int magic + exponent bias for 2^k bits
_LN2 = 0.6931471805599453
# 2^f = 1 + sum_{i>=1} EXPC[i-1] f^i  (Taylor of exp(f ln2); deg-6 err ~2e-9)
EXPC = [_LN2, _LN2**2 / 2, _LN2**3 / 6, _LN2**4 / 24, _LN2**5 / 120,
        _LN2**6 / 720]

# iface field offsets
O_RK, O_RS, O_WK, O_WS = 0, 256, 260, 324
O_ER, O_WV, O_FG, O_AG, O_WG, O_MD = 325, 389, 453, 457, 458, 459


def build_nc():
    nc = bacc.Bacc("TRN2", target_bir_lowering=False, debug=False, num_devices=8)

    co_d = nc.declare_dram_parameter("co", [T, C], F32, isOutput=False)
    w_d = nc.declare_dram_parameter("wif", [C, IF], F32, isOutput=False)
    b_d = nc.declare_dram_parameter("bif", [1, IF], F32, isOutput=False)
    m0_d = nc.declare_dram_parameter("mem0", [N, W], F32, isOutput=False)
    ident_d = nc.declare_dram_parameter("ident", [128, 128], F32, isOutput=False)
    ones_d = nc.declare_dram_parameter("ones", [128, 128], F32, isOutput=False)
    offd_d = nc.declare_dram_parameter("offdiag", [N, N], F16, isOutput=False)
    out_d = nc.declare_dram_parameter("out", [T, R, W], F32, isOutput=True)

    with tile.TileContext(nc) as tc:
        with (
            tc.tile_pool(name="const", bufs=1) as cp,
            tc.tile_pool(name="state", bufs=2) as sp,
            tc.tile_pool(name="work", bufs=2) as wp,
            tc.tile_pool(name="psP", bufs=3, space="PSUM") as psA,
            tc.tile_pool(name="psM", bufs=3, space="PSUM") as psM,
            tc.tile_pool(name="psS", bufs=1, space="PSUM") as psS,
        ):
            _build_body(nc, tc, cp, sp, wp, psA, psM, psS,
                        co_d, w_d, b_d, m0_d, ident_d, ones_d, offd_d, out_d)
    nc.compile()
    return nc


def _helpers(nc):
    V, S = nc.vector, nc.scalar

    def pexp(pool, x_ap, shape, tg, nb=1):
        """exp(x) to ~1e-7 via 2^(x*log2e): magic rounding + deg-6 poly +
        exponent-bit assembly. ~13 DVE ops; prologue/small-tensor use."""
        t_ = pool.tile(shape, F32, tag=tg + "_t", name=tg + "_t")
        V.tensor_scalar(t_[:], x_ap, LOG2E, None, op0=OP.mult)
        a_ = pool.tile(shape, F32, tag=tg + "_a", name=tg + "_a")
        V.tensor_scalar(a_[:], t_[:], MAGIC2, None, op0=OP.add)
        k_ = pool.tile(shape, F32, tag=tg + "_k", name=tg + "_k")
        V.tensor_scalar(k_[:], a_[:], MAGIC2, None, op0=OP.subtract)
        f_ = pool.tile(shape, F32, tag=tg + "_f", name=tg + "_f")
        V.tensor_tensor(f_[:], t_[:], k_[:], op=OP.subtract)
        p2 = pool.tile(shape, I32, tag=tg + "_p2", name=tg + "_p2")
        V.tensor_scalar(p2[:], a_[:].bitcast(I32), 23, None,
                        op0=OP.arith_shift_left)
        ac = [pool.tile(shape, F32, tag=tg + "_ac0", name=tg + "_ac0"),
              pool.tile(shape, F32, tag=tg + "_ac1", name=tg + "_ac1")]
        V.tensor_scalar(ac[0][:], f_[:], EXPC[5], None, op0=OP.mult)
        cur = 0
        for c_ in (EXPC[4], EXPC[3], EXPC[2], EXPC[1], EXPC[0]):
            V.scalar_tensor_tensor(ac[1 - cur][:], ac[cur][:], c_, f_[:],
                                   op0=OP.add, op1=OP.mult)
            cur = 1 - cur
        y_ = pool.tile(shape, F32, tag=tg + "_y", name=tg + "_y")
        V.scalar_tensor_tensor(y_[:], ac[cur][:], 1.0, p2[:].bitcast(F32),
                               op0=OP.add, op1=OP.mult)
        return y_

    def softplus_precise(pool, x_ap, shape, tg):
        """ln(1+e^x) with table-ln seed + one Newton step (via pexp)."""
        e_ = pexp(pool, x_ap, shape, tg + "e")
        w_ = pool.tile(shape, F32, tag=tg + "_w", name=tg + "_w")
        V.tensor_scalar(w_[:], e_[:], 1.0, None, op0=OP.add)
        z_ = pool.tile(shape, F32, tag=tg + "_z", name=tg + "_z")
        S.activation(z_[:], w_[:], AF.Ln)
        nz = pool.tile(shape, F32, tag=tg + "_nz", name=tg + "_nz")
        S.mul(nz[:], z_[:], -1.0)
        e2 = pexp(pool, nz[:], shape, tg + "e2")
        m_ = pool.tile(shape, F32, tag=tg + "_m", name=tg + "_m")
        V.tensor_tensor(m_[:], w_[:], e2[:], op=OP.mult)
        o_ = pool.tile(shape, F32, tag=tg + "_o", name=tg + "_o")
        V.scalar_tensor_tensor(o_[:], m_[:], -1.0, z_[:], op0=OP.add, op1=OP.add)
        return o_

    def rsqrt_refined(pool, x_ap, shape, tg, iters=1, nb=1):
        """rsqrt(x): ACT-table seed exp(-0.5 ln x) + Newton (no division)."""
        l_ = pool.tile(shape, F32, tag=tg + "_l", name=tg + "_l", bufs=nb)
        S.activation(l_[:], x_ap, AF.Ln)
        y_ = pool.tile(shape, F32, tag=tg + "_y", name=tg + "_y", bufs=nb)
        S.activation(y_[:], l_[:], AF.Exp, scale=-0.5)
        for i in range(iters):
            s_ = pool.tile(shape, F32, tag=tg + f"_s{i}", name=tg + f"_s{i}", bufs=nb)
            nc.gpsimd.tensor_tensor(s_[:], y_[:], y_[:], op=OP.mult)
            t_ = pool.tile(shape, F32, tag=tg + f"_t{i}", name=tg + f"_t{i}", bufs=nb)
            V.tensor_tensor(t_[:], x_ap, s_[:], op=OP.mult)
            h_ = pool.tile(shape, F32, tag=tg + f"_h{i}", name=tg + f"_h{i}", bufs=nb)
            V.tensor_scalar(h_[:], t_[:], -0.5, 1.5, op0=OP.mult, op1=OP.add)
            y2 = pool.tile(shape, F32, tag=tg + f"_y{i}", name=tg + f"_y{i}", bufs=nb)
            V.tensor_tensor(y2[:], y_[:], h_[:], op=OP.mult)
            y_ = y2
        return y_

    def rsqrt_refined_into(pool, x_ap, shape, tg, out, iters=1):
        y_ = rsqrt_refined(pool, x_ap, shape, tg, iters=iters - 1) if iters > 1 \
            else None
        if y_ is None:
            l_ = pool.tile(shape, F32, tag=tg + "_l", name=tg + "_l", bufs=1)
            S.activation(l_[:], x_ap, AF.Ln)
            y0 = pool.tile(shape, F32, tag=tg + "_y", name=tg + "_y", bufs=1)
            S.activation(y0[:], l_[:], AF.Exp, scale=-0.5)
            y_ = y0
        s_ = pool.tile(shape, F32, tag=tg + "_sf", name=tg + "_sf", bufs=1)
        nc.gpsimd.tensor_tensor(s_[:], y_[:], y_[:], op=OP.mult)
        t_ = pool.tile(shape, F32, tag=tg + "_tf", name=tg + "_tf", bufs=1)
        V.tensor_tensor(t_[:], x_ap, s_[:], op=OP.mult)
        h_ = pool.tile(shape, F32, tag=tg + "_hf", name=tg + "_hf", bufs=1)
        V.tensor_scalar(h_[:], t_[:], -0.5, 1.5, op0=OP.mult, op1=OP.add)
        V.tensor_tensor(out, y_[:], h_[:], op=OP.mult)
        return out

    def exp_refined(pool, x_ap, shape, tg, out, accum_out=None):
        """exp(x) = table seed y, then out = y*(1 + x - ln(y))."""
        y_ = pool.tile(shape, F32, tag=tg + "_y", name=tg + "_y", bufs=1)
        S.activation(y_[:], x_ap, AF.Exp)
        ly = pool.tile(shape, F32, tag=tg + "_ly", name=tg + "_ly", bufs=1)
        S.activation(ly[:], y_[:], AF.Ln)
        d_ = pool.tile(shape, F32, tag=tg + "_d", name=tg + "_d", bufs=1)
        deng = nc.gpsimd if x_ap.tensor.space == bass.MemorySpace.SBUF else V
        deng.tensor_tensor(d_[:], x_ap, ly[:], op=OP.subtract)
        V.scalar_tensor_tensor(out, d_[:], 1.0, y_[:], op0=OP.add,
                               op1=OP.mult, accum_out=accum_out)
        return out

    return pexp, softplus_precise, rsqrt_refined, exp_refined, rsqrt_refined_into


def _build_body(nc, tc, cp, sp, wp, psA, psM, psS,
                co_d, w_d, b_d, m0_d, ident_d, ones_d, offd_d, out_d,
                dbg_d=None):
    V, S, P, DMA = nc.vector, nc.scalar, nc.tensor, nc.sync
    (pexp, softplus_precise, rsqrt_refined, exp_refined,
     rsqrt_refined_into) = _helpers(nc)

    # ---------------- constants ----------------
    ident = cp.tile([128, 128], F32)
    DMA.dma_start(ident[:], ident_d[:])
    ones = cp.tile([128, 128], F32)
    DMA.dma_start(ones[:], ones_d[:])
    offd = cp.tile([128, NT, N], F16)
    for c in range(NT):
        DMA.dma_start(offd[:, c, :], offd_d[128 * c:128 * (c + 1), :])
    nege0 = cp.tile([1, N], F32)
    nc.gpsimd.memset(nege0[:], 0.0)
    nc.gpsimd.memset(nege0[0:1, 0:1], -1.0)

    # persistent per-t parse tables
    rkT = cp.tile([W, R, T], F32)
    wkT = cp.tile([W, T], F32)
    neg_er = cp.tile([W, T], F32)
    wvT = cp.tile([W, T], F32)
    bkw = cp.tile([1, T], F32)       # softplus(ws)/||wk||
    bkr = cp.tile([R, T], F32)       # softplus(rs)/||rk|| (partition-major)
    fgF = cp.tile([1, R, T], F32)
    c1n = cp.tile([1, T], F32)
    c2 = cp.tile([1, T], F32)
    modes = cp.tile([R, 3, T], F32)

    # ---------------- prologue: iface (transient pool) ----------------
    with tc.tile_pool(name="prolog", bufs=1) as pp:
        co_sb = pp.tile([T, C], F32)
        DMA.dma_start(co_sb[:], co_d[:])
        bif_sb = pp.tile([1, IF], F32)
        DMA.dma_start(bif_sb[:], b_d[:])

        coT_p = psA.tile([128, 8, T], F32, tag="p")
        for k in range(8):
            P.transpose(coT_p[:, k, :], co_sb[:, 128 * k:128 * (k + 1)],
                        ident[0:T, 0:T])
        coT = pp.tile([128, 8, T], F32)
        S.copy(coT[:], coT_p[:])

        if_p = psS.tile([T, IF], F32, tag="s")
        for h in range(2):
            w_sb = pp.tile([128, 4, IF], F32, tag="w_sb", name=f"w_sb{h}")
            for k in range(4):
                DMA.dma_start(w_sb[:, k, :],
                              w_d[128 * (4 * h + k):128 * (4 * h + k + 1), :])
            for k in range(4):
                P.matmul(if_p[:], coT[:, 4 * h + k, :], w_sb[:, k, :],
                         start=(h == 0 and k == 0), stop=False)
        P.matmul(if_p[:], ones[0:1, 0:T], bif_sb[:], start=False, stop=True)
        iface = pp.tile([T, IF], F32)
        S.copy(iface[:], if_p[:])

        # field transposes -> per-t column layouts
        def tp_field(lo, hi):
            n = hi - lo
            pt = psA.tile([128, T], F32, tag="p")
            P.transpose(pt[0:n, :], iface[:, lo:hi], ident[0:T, 0:T])
            return pt

        for r in range(R):
            pt = tp_field(O_RK + W * r, O_RK + W * (r + 1))
            S.copy(rkT[:, r, :], pt[0:W, :])
        pt = tp_field(O_WK, O_WK + W)
        S.copy(wkT[:], pt[0:W, :])
        pt = tp_field(O_ER, O_ER + W)
        er_in = pp.tile([W, T], F32)
        V.tensor_scalar(er_in[:], pt[0:W, :], -1.0, None, op0=OP.mult)
        er_e = pexp(pp, er_in[:], [W, T], "sge")     # e^{-x}
        er_w = pp.tile([W, T], F32)
        V.tensor_scalar(er_w[:], er_e[:], 1.0, None, op0=OP.add)
        er_r = pp.tile([W, T], F32)
        V.reciprocal(er_r[:], er_w[:])               # sigmoid(x)
        V.tensor_scalar(neg_er[:], er_r[:], -1.0, None, op0=OP.mult)
        pt = tp_field(O_WV, O_WV + W)
        S.copy(wvT[:], pt[0:W, :])

        # strengths / ||k|| folded: bkw, bkr  (free-layout pipeline)
        rsF_p = psA.tile([1, R, T], F32, tag="p")
        for r in range(R):
            P.transpose(rsF_p[0:1, r, :], iface[:, O_RS + r:O_RS + r + 1],
                        ident[0:T, 0:T])
        rsF = pp.tile([1, R, T], F32)
        S.copy(rsF[:], rsF_p[:])
        wsF_p = psA.tile([1, T], F32, tag="p")
        P.transpose(wsF_p[:], iface[:, O_WS:O_WS + 1], ident[0:T, 0:T])
        wsF = pp.tile([1, T], F32)
        S.copy(wsF[:], wsF_p[:])
        rs_pre = softplus_precise(pp, rsF[:].rearrange("o r t -> o (r t)"),
                                  [1, R * T], "rsp")
        ws_pre = softplus_precise(pp, wsF[:], [1, T], "wsp")

        sqw = pp.tile([W, T], F32)
        S.square(sqw[:], wkT[:])
        wk2_p = psM.tile([1, T], F32, tag="m")
        P.matmul(wk2_p[:], ones[0:W, 0:1], sqw[:])
        wkr = rsqrt_refined(pp, wk2_p[:], [1, T], "wkr", iters=2)
        V.tensor_tensor(bkw[:], ws_pre[:], wkr[:], op=OP.mult)

        sqr = pp.tile([W, R, T], F32)
        S.square(sqr[:], rkT[:])
        rk2_p = psM.tile([1, R * T], F32, tag="m")
        P.matmul(rk2_p[:], ones[0:W, 0:1], sqr[:].rearrange("w r t -> w (r t)"))
        rkr = rsqrt_refined(pp, rk2_p[:], [1, R * T], "rkr", iters=2)
        bkrF = pp.tile([1, R, T], F32)
        V.tensor_tensor(bkrF[:].rearrange("o r t -> o (r t)"), rs_pre[:],
                        rkr[:], op=OP.mult)
        bkr_p = psA.tile([R, T], F32, tag="p")
        for t in range(T):
            P.transpose(bkr_p[:, t:t + 1], bkrF[0:1, :, t], ident[0:1, 0:1])
        S.copy(bkr[:], bkr_p[:])

        # gates
        # fg, ag, wg sigmoids via precise V pipeline, packed in one [1,6,T]
        gats_p = psA.tile([1, 6, T], F32, tag="p")
        for r in range(R):
            P.transpose(gats_p[0:1, r, :], iface[:, O_FG + r:O_FG + r + 1],
                        ident[0:T, 0:T])
        P.transpose(gats_p[0:1, 4, :], iface[:, O_AG:O_AG + 1], ident[0:T, 0:T])
        P.transpose(gats_p[0:1, 5, :], iface[:, O_WG:O_WG + 1], ident[0:T, 0:T])
        g_in = pp.tile([1, 6 * T], F32)
        V.tensor_scalar(g_in[:], gats_p[:].rearrange("o g t -> o (g t)"), -1.0,
                        None, op0=OP.mult)
        g_e = pexp(pp, g_in[:], [1, 6 * T], "sgg")
        g_w = pp.tile([1, 6 * T], F32)
        V.tensor_scalar(g_w[:], g_e[:], 1.0, None, op0=OP.add)
        g_r = pp.tile([1, 6, T], F32)
        V.reciprocal(g_r[:].rearrange("o g t -> o (g t)"), g_w[:])
        V.tensor_copy(fgF[:], g_r[0:1, 0:R, :])
        ag_t = g_r[0:1, 4, :]
        wg_t = g_r[0:1, 5, :]
        c1t = pp.tile([1, T], F32)
        V.tensor_tensor(c1t[:], ag_t, wg_t, op=OP.mult)
        V.tensor_scalar(c1n[:], c1t[:], -1.0, None, op0=OP.mult)
        V.tensor_tensor(c2[:], wg_t, c1t[:], op=OP.subtract)

        # modes softmax (precise exp; normalize in [T,12]; m-major; transpose)
        me = pexp(pp, iface[:, O_MD:O_MD + 12], [T, 12], "me")
        me3 = me[:].rearrange("t (r m) -> t r m", m=3)
        msum = pp.tile([T, R], F32)
        V.tensor_tensor(msum[:], me3[:, :, 0], me3[:, :, 1], op=OP.add)
        V.tensor_tensor(msum[:], msum[:], me3[:, :, 2], op=OP.add)
        mrs = pp.tile([T, R], F32)
        V.reciprocal(mrs[:], msum[:])
        mn = pp.tile([T, 12], F32)
        mn3 = mn[:].rearrange("t (r m) -> t r m", m=3)
        for m in range(3):
            V.tensor_tensor(mn3[:, :, m], me3[:, :, m], mrs[:], op=OP.mult)
        mo = pp.tile([T, 12], F32)
        mo3 = mo[:].rearrange("t (m r) -> t m r", r=R)
        S.copy(mo3[:], mn3[:].rearrange("t r m -> t m r"))
        modes_p = psA.tile([R, 3, T], F32, tag="p")
        for m in range(3):
            P.transpose(modes_p[:, m, :], mo[:, 4 * m:4 * (m + 1)],
                        ident[0:T, 0:T])
        S.copy(modes[:], modes_p[:])

    # ---------------- initial state ----------------
    mem_nrm = sp.tile([128, NT, W], F32, tag="mem_nrm")
    for c in range(NT):
        DMA.dma_start(mem_nrm[:, c, :], m0_d[128 * c:128 * (c + 1), :])
    memT_p = psA.tile([W, N], F32, tag="p")
    for c in range(NT):
        P.transpose(memT_p[:, 128 * c:128 * (c + 1)], mem_nrm[:, c, :], ident[:])
    memT = sp.tile([W, N], F32, tag="memT")
    S.copy(memT[:], memT_p[:])

    sqm = wp.tile([W, N], F32, tag="sqm", bufs=1)
    S.square(sqm[:], memT[:])
    ms_p = psM.tile([1, N], F32, tag="m")
    P.matmul(ms_p[:], ones[0:W, 0:1], sqm[:])
    w_rs = rsqrt_refined(wp, ms_p[:], [1, N], "w1", iters=1)
    mnorm = sp.tile([1, N], F32, tag="mnorm")
    V.tensor_copy(mnorm[:], w_rs[:])

    L = sp.tile([128, NT, N], F32, tag="L")
    nc.gpsimd.memset(L[:], 0.0)
    LT0 = sp.tile([128, NT, N], F32, tag="LT")
    nc.gpsimd.memset(LT0[:], 0.0)
    u0 = sp.tile([1, N], F32, tag="u")
    nc.gpsimd.memset(u0[:], 0.0)
    rw0 = sp.tile([R, N], F32, tag="rw")
    nc.gpsimd.memset(rw0[:], 0.0)
    rwT0 = sp.tile([128, NT * R], F32, tag="rwT")
    nc.gpsimd.memset(rwT0[:], 0.0)

    out_sb = cp.tile([R, T, W], F32)
    dbg_sb = None

    st = dict(memT=memT, mem_nrm=mem_nrm, mnorm=mnorm, L=L, LT=LT0,
              u=u0, prec=None, rw=rw0, rwT=rwT0)

    for t in range(T):
        st = _step(nc, t, st, cp, sp, wp, psA, psM, psS,
                   ident, ones, offd, nege0, rkT, wkT, neg_er, wvT,
                   bkw, bkr, fgF, c1n, c2, modes, out_sb,
                   rsqrt_refined, exp_refined, rsqrt_refined_into)

    DMA.dma_start(out_d[:].rearrange("t r w -> r t w"), out_sb[:])


def _step(nc, t, st, cp, sp, wp, psA, psM, psS,
          ident, ones, offd, nege0, rkT, wkT, neg_er, wvT,
          bkw, bkr, fgF, c1n, c2, modes, out_sb,
          rsqrt_refined, exp_refined, rsqrt_refined_into, dbg_sb=None):
    V, S, P = nc.vector, nc.scalar, nc.tensor
    memT, mem_nrm, mnorm = st["memT"], st["mem_nrm"], st["mnorm"]
    L, LT, u, prec, rw, rwT = st["L"], st["LT"], st["u"], st["prec"], st["rw"], st["rwT"]
    last = (t == T - 1)

    # ---- write content weights (on memory entering the step) ----
    # mnorm state holds rsqrt(sum mem^2); bkw = softplus(ws)/||wk||.
    wdots_p = psM.tile([1, N], F32, tag="m")
    P.matmul(wdots_p[:], wkT[:, t:t + 1], memT[:])
    wlog = wp.tile([1, N], F32, tag="wlog")
    V.scalar_tensor_tensor(wlog[:], wdots_p[:], bkw[0:1, t:t + 1],
                           mnorm[0:1, :], op0=OP.mult, op1=OP.mult)
    wsum = wp.tile([1, 1], F32, tag="wsum")
    wexp = wp.tile([1, N], F32, tag="wexp")
    exp_refined(wp, wlog[:], [1, N], "wex", wexp[:], accum_out=wsum[:])
    wrs = wp.tile([1, 1], F32, tag="wrs")
    V.reciprocal(wrs[:], wsum[:])

    # ---- allocation weighting ----
    if t == 0:
        negalloc = nege0
        omu_a = None
    else:
        u_tp = psA.tile([128, NT], F32, tag="p")
        for c in range(NT):
            P.transpose(u_tp[:, c:c + 1], u[0:1, 128 * c:128 * (c + 1)],
                        ident[0:1, 0:1])
        u_pm = wp.tile([128, NT], F32, tag="u_pm")
        V.tensor_copy(u_pm[:], u_tp[:])
        lu_pm = wp.tile([128, NT], F32, tag="lu_pm")
        S.activation(lu_pm[:], u_pm[:], AF.Ln)
        ub_p = psA.tile([128, N], F32, tag="p")
        P.matmul(ub_p[:], ones[0:1, :], u[:])
        G = wp.tile([128, NT, N], F32, tag="G", bufs=1)
        for c in range(NT):
            V.tensor_scalar(G[:, c, :], ub_p[:], u_pm[:, c:c + 1], None, op0=OP.is_gt)
        s_p = psM.tile([1, N], F32, tag="m")
        for c in range(NT):
            P.matmul(s_p[:], lu_pm[:, c:c + 1], G[:, c, :],
                     start=(c == 0), stop=(c == NT - 1))
        s_c = wp.tile([1, N], F32, tag="s_c")
        V.tensor_scalar(s_c[:], s_p[:], -80.0, None, op0=OP.max)
        es = wp.tile([1, N], F32, tag="es")
        exp_refined(wp, s_c[:], [1, N], "wex", es[:])
        omu_a = wp.tile([1, N], F32, tag="omu")
        V.tensor_scalar(omu_a[:], u[:], -1.0, 1.0, op0=OP.mult, op1=OP.add)
        negalloc = wp.tile([1, N], F32, tag="negalloc")
        V.scalar_tensor_tensor(negalloc[:], omu_a[:], -1.0, es[:],
                               op0=OP.mult, op1=OP.mult)

    # ---- write weights ww ----
    c2rs = wp.tile([1, 1], F32, tag="c2rs")
    V.tensor_tensor(c2rs[:], wrs[:], c2[0:1, t:t + 1], op=OP.mult)
    t_wc = wp.tile([1, N], F32, tag="t_wc")
    V.tensor_scalar(t_wc[:], wexp[:], c2rs[:], None, op0=OP.mult)
    ww = wp.tile([1, N], F32, tag="ww")
    sw = wp.tile([1, 1], F32, tag="sw")
    V.scalar_tensor_tensor(ww[:], negalloc[:], c1n[0:1, t:t + 1], t_wc[:],
                           op0=OP.mult, op1=OP.add, accum_out=sw[:])

    # ---- prec update (uses prec BEFORE update; link also uses old prec) ----
    if t == 0:
        prec_n = ww  # (1-sw)*0 + ww
    elif last:
        prec_n = None
    else:
        omsw = wp.tile([1, 1], F32, tag="omsw")
        V.tensor_scalar(omsw[:], sw[:], -1.0, 1.0, op0=OP.mult, op1=OP.add)
        prec_n = sp.tile([1, N], F32, tag="prec")
        V.scalar_tensor_tensor(prec_n[:], prec[:], omsw[:], ww[:],
                               op0=OP.mult, op1=OP.add)

    # ---- usage update ----
    if t == 0:
        u_n = ww  # psi=1, u=0 -> u' = ww
    elif last:
        u_n = None
    else:
        fgb_p = psA.tile([128, R], F32, tag="p")
        P.matmul(fgb_p[:], ones[0:1, :], fgF[0:1, :, t])
        yyT = wp.tile([128, NT, R], F32, tag="yyT")
        V.scalar_tensor_tensor(
            yyT[:], fgb_p[:, None, :].broadcast_to([128, NT, R]), -1.0,
            rwT[:].rearrange("p (c r) -> p c r", r=R), op0=OP.mult, op1=OP.mult)
        om = wp.tile([128, NT, R], F32, tag="om")
        V.tensor_scalar(om[:], yyT[:], 1.0, None, op0=OP.add)
        p1 = wp.tile([128, NT], F32, tag="p1")
        V.tensor_tensor(p1[:], om[:, :, 0], om[:, :, 1], op=OP.mult)
        p2 = wp.tile([128, NT], F32, tag="p2")
        V.tensor_tensor(p2[:], om[:, :, 2], om[:, :, 3], op=OP.mult)
        psi_pm = wp.tile([128, NT], F32, tag="psi_pm")
        V.tensor_tensor(psi_pm[:], p1[:], p2[:], op=OP.mult)
        psiT_p = psA.tile([1, N], F32, tag="p")
        for c in range(NT):
            P.transpose(psiT_p[0:1, 128 * c:128 * (c + 1)], psi_pm[:, c:c + 1],
                        ident[:])
        tn = wp.tile([1, N], F32, tag="tn")
        V.scalar_tensor_tensor(tn[:], ww[:], 1.0, omu_a[:],
                               op0=OP.subtract, op1=OP.mult)
        u_n = sp.tile([1, N], F32, tag="u")
        V.scalar_tensor_tensor(u_n[:], tn[:], 1.0, psiT_p[:],
                               op0=OP.add, op1=OP.mult)

    # ---- memory update ----
    wwb_p = psM.tile([W, N], F32, tag="m")
    P.matmul(wwb_p[:], ones[0:1, 0:W], ww[:])
    keep = wp.tile([W, N], F32, tag="keep", bufs=1)
    V.tensor_scalar(keep[:], wwb_p[:], neg_er[:, t:t + 1], 1.0,
                    op0=OP.mult, op1=OP.add)
    m1 = wp.tile([W, N], F32, tag="m1", bufs=1)
    nc.gpsimd.tensor_tensor(m1[:], memT[:], keep[:], op=OP.mult)
    memT_n = sp.tile([W, N], F32, tag="memT")
    V.scalar_tensor_tensor(memT_n[:], wwb_p[:], wvT[:, t:t + 1], m1[:],
                           op0=OP.mult, op1=OP.add)
    mem_nrm_p = psA.tile([128, NT, W], F32, tag="p")
    for c in range(NT):
        P.transpose(mem_nrm_p[:, c, :], memT_n[:, 128 * c:128 * (c + 1)],
                    ident[0:W, 0:W])
    mem_nrm_n = sp.tile([128, NT, W], F32, tag="mem_nrm")
    V.tensor_copy(mem_nrm_n[:], mem_nrm_p[:])
    sqm = wp.tile([W, N], F32, tag="sqm", bufs=1)
    nc.gpsimd.tensor_tensor(sqm[:], memT_n[:], memT_n[:], op=OP.mult)
    ms4_p = psM.tile([R, N], F32, tag="m")
    P.matmul(ms4_p[:], ones[0:W, 0:R], sqm[:])
    mnorm_n = sp.tile([R, N], F32, tag="mnorm")
    rsqrt_refined_into(wp, ms4_p[:], [R, N], "w1", mnorm_n[:], iters=1)

    # ---- link update ----
    if t == 0:
        L_n, LT_n = L, LT  # stays zero
    else:
        ww_tp = psA.tile([128, NT], F32, tag="p")
        for c in range(NT):
            P.transpose(ww_tp[:, c:c + 1], ww[0:1, 128 * c:128 * (c + 1)],
                        ident[0:1, 0:1])
        w_pm = wp.tile([128, NT], F32, tag="w_pm")
        V.tensor_copy(w_pm[:], ww_tp[:])
        omw_pm = wp.tile([128, NT], F32, tag="omw_pm")
        V.tensor_scalar(omw_pm[:], w_pm[:], -1.0, 1.0, op0=OP.mult, op1=OP.add)
        wb_p = psA.tile([128, N], F32, tag="p")
        P.matmul(wb_p[:], ones[0:1, :], ww[:])
        pb_p = psA.tile([128, N], F32, tag="p")
        P.matmul(pb_p[:], ones[0:1, :], prec[:])
        L_n = sp.tile([128, NT, N], F32, tag="L")
        for c in range(NT):
            pbm = wp.tile([128, N], F32, tag="pbm")
            V.tensor_tensor(pbm[:], pb_p[:], offd[:, c, :], op=OP.mult)
            t1 = wp.tile([128, N], F32, tag="t1")
            V.scalar_tensor_tensor(t1[:], wb_p[:], omw_pm[:, c:c + 1], L[:, c, :],
                                   op0=OP.subtract, op1=OP.mult)
            V.scalar_tensor_tensor(L_n[:, c, :], pbm[:], w_pm[:, c:c + 1], t1[:],
                                   op0=OP.mult, op1=OP.subtract)
        LT_n = sp.tile([128, NT, N], F32, tag="LT")
        for j in range(NT):
            lt_p = psA.tile([128, N], F32, tag="p")
            for i in range(NT):
                P.transpose(lt_p[:, 128 * i:128 * (i + 1)],
                            L_n[:, i, 128 * j:128 * (j + 1)], ident[:])
            V.tensor_copy(LT_n[:, j, :], lt_p[:])

    # ---- read content weights (on updated memory) ----
    rdots_p = psM.tile([R, N], F32, tag="m")
    P.matmul(rdots_p[:], rkT[:, :, t], memT_n[:])
    rlog = wp.tile([R, N], F32, tag="rlog")
    V.scalar_tensor_tensor(rlog[:], rdots_p[:], bkr[:, t:t + 1], mnorm_n[:],
                           op0=OP.mult, op1=OP.mult)
    rsum = wp.tile([R, 1], F32, tag="rsum")
    rexp = wp.tile([R, N], F32, tag="rexp")
    exp_refined(wp, rlog[:], [R, N], "rex", rexp[:], accum_out=rsum[:])
    rsr = wp.tile([R, 1], F32, tag="rsr")
    V.reciprocal(rsr[:], rsum[:])
    s1c = wp.tile([R, 1], F32, tag="s1c")
    V.tensor_tensor(s1c[:], rsr[:], modes[:, 1, t:t + 1], op=OP.mult)

    # ---- read weights ----
    rw_n = sp.tile([R, N], F32, tag="rw")
    if t == 0:
        V.tensor_scalar(rw_n[:], rexp[:], s1c[:], None, op0=OP.mult)
    else:
        bwd_p = psM.tile([R, N], F32, tag="m")
        for c in range(NT):
            P.matmul(bwd_p[:], rwT[:, R * c:R * (c + 1)], L_n[:, c, :],
                     start=(c == 0), stop=(c == NT - 1))
        fwd_p = psM.tile([R, N], F32, tag="m")
        for c in range(NT):
            P.matmul(fwd_p[:], rwT[:, R * c:R * (c + 1)], LT_n[:, c, :],
                     start=(c == 0), stop=(c == NT - 1))
        a1 = wp.tile([R, N], F32, tag="a1")
        V.tensor_scalar(a1[:], rexp[:], s1c[:], None, op0=OP.mult)
        b1 = wp.tile([R, N], F32, tag="b1")
        V.scalar_tensor_tensor(b1[:], fwd_p[:], modes[:, 2, t:t + 1], a1[:],
                               op0=OP.mult, op1=OP.add)
        V.scalar_tensor_tensor(rw_n[:], bwd_p[:], modes[:, 0, t:t + 1], b1[:],
                               op0=OP.mult, op1=OP.add)

    rwT_p = psA.tile([128, NT * R], F32, tag="p")
    for c in range(NT):
        P.transpose(rwT_p[:, R * c:R * (c + 1)], rw_n[:, 128 * c:128 * (c + 1)],
                    ident[0:R, 0:R])
    rwT_n = sp.tile([128, NT * R], F32, tag="rwT")
    V.tensor_copy(rwT_n[:], rwT_p[:])

    # ---- read words ----
    rwd_p = psS.tile([R, W], F32, tag="s")
    for c in range(NT):
        P.matmul(rwd_p[:], rwT_n[:, R * c:R * (c + 1)], mem_nrm_n[:, c, :],
                 start=(c == 0), stop=(c == NT - 1))
    V.tensor_copy(out_sb[:, t, :], rwd_p[:])

    return dict(memT=memT_n, mem_nrm=mem_nrm_n, mnorm=mnorm_n, L=L_n, LT=LT_n,
                u=u_n, prec=prec_n, rw=rw_n, rwT=rwT_n)


# ---------------------------------------------------------------------------
_NC_CACHE = {}


def _get_nc():
    if "nc" not in _NC_CACHE:
        _NC_CACHE["nc"] = build_nc()
    return _NC_CACHE["nc"]


def _consts():
    ident = np.eye(128, dtype=np.float32)
    ones = np.ones((128, 128), dtype=np.float32)
    offd = (1.0 - np.eye(N)).astype(np.float16)
    return ident, ones, offd


def make_in_maps(controller_output, W_if, b_if, memory0):
    ident, ones, offd = _consts()
    maps = []
    for b in range(B):
        maps.append({
            "co": np.ascontiguousarray(controller_output[b]),
            "wif": np.ascontiguousarray(W_if),
            "bif": np.ascontiguousarray(b_if.reshape(1, IF)),
            "mem0": np.ascontiguousarray(memory0[b]),
            "ident": ident, "ones": ones, "offdiag": offd,
        })
    return maps


def kernel(controller_output, W_if, b_if, memory0):
    from concourse.bass_utils import run_bass_kernel_spmd
    controller_output = np.asarray(controller_output, dtype=np.float32)
    W_if = np.asarray(W_if, dtype=np.float32)
    b_if = np.asarray(b_if, dtype=np.float32)
    memory0 = np.asarray(memory0, dtype=np.float32)
    nc = _get_nc()
    maps = make_in_maps(controller_output, W_if, b_if, memory0)
    res = run_bass_kernel_spmd(nc, maps, core_ids=list(range(B)))
    return np.stack([res.results[b]["out"] for b in range(B)], axis=0)


if __name__ == "__main__":
    mode = sys.argv[1] if len(sys.argv) > 1 else "sim"
    sys.path.insert(0, "/root/problem")
    import jax
    with jax.default_device(jax.devices("cpu")[0]):
        import reference
        inputs = {k: np.asarray(v) for k, v in reference.setup_inputs().items()}
        expected = np.asarray(reference.reference(**inputs))

    if mode == "sim":
        from concourse.bass_interp import CoreSim
        nc = build_nc()
        maps = make_in_maps(inputs["controller_output"], inputs["W_if"],
                            inputs["b_if"], inputs["memory0"])
        sim = CoreSim(nc)
        for k, v in maps[0].items():
            sim.tensor(k)[:] = v
        sim.simulate()
        got = sim.tensor("out").copy()
        exp = expected[0]
        err = np.abs(got - exp)
        rel = np.linalg.norm(got - exp) / (np.linalg.norm(exp) + 1e-12)
        print("sim modeled time (ns):", sim.time)
        print("max abs err:", err.max(), " rel err:", rel)
    else:
        got = kernel(**inputs)
        rel = np.linalg.norm(got - expected) / (np.linalg.norm(expected) + 1e-12)
        print("max abs err:", np.abs(got - expected).max(), " rel err:", rel)
